# revision 1
# baseline (speedup 1.0000x reference)
"""Deformable transformer encoder layer on 8 Trainium2 NeuronCores.

Sharding: core c handles batch c//4, query-quarter c%4 (3840 queries each).
Per core:
  - value = src @ w_val + b_val over the core's full batch, written bf16 to a
    level-padded HBM table (zero pad rows between levels absorb out-of-level
    taps with zero weight, matching the reference's validity masking).
  - sampling: per (query-tile, level) one indirect DMA gathers a 6-row x 256
    column window starting at floor(ref*T - 0.5) - 2; bilinear tap weights
    become window-row coefficients via the hat identity
    w0*d(r-j0) + w1*d(r-j0-1) == max(0, 1 - |r - x|),  x = a - base,
    computed arithmetically (no floor of per-tap positions needed); a PE
    matmul against a constant expansion matrix broadcasts the coefficients
    over the 32 dims of each head; DVE multiplies and tree-reduces.
  - projections / FFN / layernorms: bf16 PE matmuls, PE transposes, biases
    folded into K=1 ones-row matmuls or ACT per-partition bias.
"""
import os
import sys

sys.path.insert(0, '/opt/trn_rl_repo')

import dataclasses
import numpy as np
import ml_dtypes

import concourse.bass as bass
import concourse.mybir as mybir
from concourse.tile import TileContext

# ---- tile drain workaround (this walrus rejects multi-wait drains) ----
import concourse.tile as _tile_mod
from concourse.tile_sem_assignment import tick_to_sem as _tick_to_sem


def _split_drain_and_barrier(self, tick_clock, wait_clock):
    gc = tick_clock.global_clock
    allocated = self.sems.allocated() if self.sems is not None else {}
    for proc, sem in sorted(allocated.items()):
        t = gc[proc]
        if t > 0:
            self.nc.sync.wait_ge(sem, _tick_to_sem(t, proc))
    self.nc.sync.drain()
    self.nc.all_engine_barrier()
    assert self.sems is not None
    popped = self.nc._tile_sem_poison_stack.pop()
    assert popped is self._sem_poison
    self.nc.clear_and_free_semaphores(list(self.sems.allocated().values()))
    self.nc.all_engine_barrier()


_tile_mod.TileContext._drain_and_barrier = _split_drain_and_barrier

_MAX_WAITS = 1
_wsplit_n = [0]


def _split_excess_waits(nc):
    """Walrus rejects instructions with >2 sem waits; move extras to nops."""
    for f in nc.m.functions:
        for bb in f.blocks:
            out = []
            for inst in list(bb.instructions):
                si = inst.sync_info
                waits = list(si.on_wait) if (si and si.on_wait) else []
                if len(waits) > _MAX_WAITS:
                    extra = waits[:-_MAX_WAITS]
                    keep = waits[-_MAX_WAITS:]
                    for j in range(0, len(extra), _MAX_WAITS):
                        _wsplit_n[0] += 1
                        nop = mybir.InstNoOp(name=f'wsplit-{_wsplit_n[0]}',
                                             ins=[], outs=[])
                        nop.engine = inst.engine
                        nop.sync_info = mybir.SyncInfo(
                            on_wait=extra[j:j + _MAX_WAITS], on_update=[])
                        out.append(nop)
                    inst.sync_info = mybir.SyncInfo(
                        on_wait=keep, on_update=list(si.on_update or []))
                out.append(inst)
            bb.instructions = out

f32 = mybir.dt.float32
bf16 = mybir.dt.bfloat16
i32 = mybir.dt.int32
AF = mybir.ActivationFunctionType
OP = mybir.AluOpType

# ---- problem constants ----
D = 256
DF = 1024
H = 8
L = 4
NP = 4
LEVEL_LENS = (8192, 4096, 2048, 1024)
LEN_IN = 15360
N_CORES = 8
EPS = 1e-5

W = 6
PAD = 8
_starts = []
_acc = PAD
for _t in LEVEL_LENS:
    _starts.append(_acc)
    _acc += _t + PAD
PSTARTS = tuple(_starts)
VROWS = _acc               # 15400
QC = LEN_IN // 4           # 3840
NT_V = LEN_IN // 128       # 120
NT_Q = QC // 128           # 30
WIN = W * D                # 1536
HLP = H * L * NP           # 128


def _bc(ap, dims):
    """Replace the free dims of a 2-d AP with an explicit dim list."""
    return dataclasses.replace(ap, ap=[list(ap.ap[0])] + [list(d) for d in dims])


def _build_program():
    nc = bass.Bass(trn_type='TRN2')

    din = {}
    def I(name, shape, dt):
        din[name] = nc.dram_tensor(name, shape, dt, kind='ExternalInput')
        return din[name]

    src_full = I('src_full', [LEN_IN, D], f32)
    srcq = I('srcq', [QC, D], f32)
    pos_q = I('pos_q', [QC, D], f32)
    te_q = I('te_q', [QC, D], f32)
    ref_q = I('ref_q', [QC, L], f32)
    wval = I('wval', [128, 2 * D], bf16)     # chunk kc at cols [kc*D,(kc+1)*D]
    bval = I('bval', [1, D], bf16)
    woa = I('woa', [128, 2 * D], bf16)
    boa = I('boa', [1, D], bf16)
    wout = I('wout', [128, 2 * D], bf16)
    bout = I('bout', [1, D], bf16)
    w1 = I('w1', [128, 2 * DF], bf16)        # chunk kc at cols [kc*DF,(kc+1)*DF]
    b1t = I('b1t', [128, 8], f32)            # b1 transposed: [dffn%128, dffn//128]
    w2 = I('w2', [128, 8 * D], bf16)         # chunk k at cols [k*D,(k+1)*D]
    b2 = I('b2', [1, D], bf16)
    ln1g = I('ln1g', [128, D], f32)          # replicated over partitions
    ln1b = I('ln1b', [128, D], f32)
    ln2g = I('ln2g', [128, D], f32)
    ln2b = I('ln2b', [128, D], f32)
    ident = I('ident', [128, 128], bf16)
    ones1 = I('ones1', [1, 128], bf16)
    emat = I('emat', [H * W, WIN], bf16)
    riota = I('riota', [128, HLP * W], f32)  # replicated; col (hlp)*W+r -> r
    tsc = I('tsc', [128, L], f32)            # replicated level lens
    slc = I('slc', [128, L], f32)            # replicated PSTARTS[l] - 18
    boffr = I('boffr', [128, HLP], f32)      # replicated b_off over partitions

    out_q = nc.dram_tensor('out_q', [QC, D], f32, kind='ExternalOutput')

    with TileContext(nc) as tc:
        with tc.tile_pool(name='cst', bufs=1) as cst, \
             tc.tile_pool(name='io', bufs=3) as io, \
             tc.tile_pool(name='wk', bufs=3) as wk, \
             tc.tile_pool(name='gat', bufs=4) as gat, \
             tc.tile_pool(name='p256', bufs=3, space='PSUM') as p256, \
             tc.tile_pool(name='ptr', bufs=4, space='PSUM') as ptr, \
             tc.tile_pool(name='dram', bufs=1, space='DRAM') as dram:

            value_dram = dram.tile([VROWS, D], bf16)

            def ctile(name, dt=bf16):
                t = cst.tile(list(din[name].shape), dt, tag=name)
                nc.sync.dma_start(t[:], din[name][:, :])
                return t

            identS = ctile('ident')
            onesS = ctile('ones1')
            ematS = ctile('emat')
            riotaS = ctile('riota', f32)
            tscS = ctile('tsc', f32)
            slcS = ctile('slc', f32)
            boffrS = ctile('boffr', f32)
            wvalS = ctile('wval')
            bvalS = ctile('bval')
            woaS = ctile('woa')
            boaS = ctile('boa')
            woutS = ctile('wout')
            boutS = ctile('bout')
            w1S = ctile('w1')
            b1tS = ctile('b1t', f32)
            w2S = ctile('w2')
            b2S = ctile('b2')
            ln1gS = ctile('ln1g', f32)
            ln1bS = ctile('ln1b', f32)
            ln2gS = ctile('ln2g', f32)
            ln2bS = ctile('ln2b', f32)

            # ---- zero pad rows of value table ----
            epsS = cst.tile([128, 1], f32, tag='epsS')
            nc.vector.memset(epsS[:], EPS)
            zpad = cst.tile([PAD, D], bf16, tag='zpad')
            nc.vector.memset(zpad[:], 0.0)
            nc.sync.dma_start(value_dram[0:PAD, :], zpad[:])
            for lv in range(L):
                r0 = PSTARTS[lv] + LEVEL_LENS[lv]
                nc.sync.dma_start(value_dram[r0:r0 + PAD, :], zpad[:])

            # ---- phase A: value projection ----
            cums = [0]
            for t in LEVEL_LENS:
                cums.append(cums[-1] + t)
            for i in range(NT_V):
                r = i * 128
                lv = next(k for k in range(L) if r < cums[k + 1])
                prow = PSTARTS[lv] + (r - cums[lv])

                s_t = io.tile([128, D], f32, tag='va_in')
                nc.sync.dma_start(s_t[:], src_full[r:r + 128, :])
                s_b = wk.tile([128, D], bf16, tag='va_b')
                nc.scalar.activation(s_b[:], s_t[:], AF.Copy)
                vps = p256.tile([128, D], f32, tag='p256')
                for kc in range(2):
                    tp = ptr.tile([128, 128], bf16, tag='tr')
                    nc.tensor.transpose(tp[:], s_b[:, kc * 128:(kc + 1) * 128], identS[:])
                    tb = wk.tile([128, 128], bf16, tag='va_trb')
                    nc.vector.tensor_copy(tb[:], tp[:])
                    nc.tensor.matmul(vps[:], tb[:], wvalS[:, kc * D:(kc + 1) * D],
                                     start=(kc == 0), stop=False)
                nc.tensor.matmul(vps[:], onesS[:1, :], bvalS[:1, :], start=False, stop=True)
                v_b = wk.tile([128, D], bf16, tag='va_out')
                nc.vector.tensor_copy(v_b[:], vps[:])
                nc.sync.dma_start(value_dram[prow:prow + 128, :], v_b[:])

            vwin = dataclasses.replace(value_dram[:, :], ap=[[D, VROWS - W], [1, D]])

            # ---- phase B/C/D ----
            for i in range(int(os.environ.get('K_NTQ', NT_Q))):
                rq = i * 128
                srco = io.tile([128, D], f32, tag='srco')
                nc.sync.dma_start(srco[:], srcq[rq:rq + 128, :])
                post = io.tile([128, D], f32, tag='post')
                nc.sync.dma_start(post[:], pos_q[rq:rq + 128, :])
                tet = io.tile([128, D], f32, tag='tet')
                nc.sync.dma_start(tet[:], te_q[rq:rq + 128, :])
                reft = io.tile([128, L], f32, tag='reft')
                nc.sync.dma_start(reft[:], ref_q[rq:rq + 128, :])

                # q = src + pos -> bf16 -> transpose
                qf = wk.tile([128, D], f32, tag='qf')
                nc.vector.tensor_tensor(qf[:], srco[:], post[:], OP.add)
                qb = wk.tile([128, D], bf16, tag='qb')
                nc.scalar.activation(qb[:], qf[:], AF.Copy)
                qT = wk.tile([128, D], bf16, tag='qT')
                for kc in range(2):
                    tp = ptr.tile([128, 128], bf16, tag='tr')
                    nc.tensor.transpose(tp[:], qb[:, kc * 128:(kc + 1) * 128], identS[:])
                    nc.scalar.activation(qT[:, kc * 128:(kc + 1) * 128], tp[:], AF.Copy)

                # off/attn projection (+bias)
                oaps = p256.tile([128, D], f32, tag='p256')
                nc.tensor.matmul(oaps[:], qT[:, 0:128], woaS[:, 0:D], start=True, stop=False)
                nc.tensor.matmul(oaps[:], qT[:, 128:256], woaS[:, D:2 * D], start=False, stop=True)

                # softmax pieces
                ex = wk.tile([128, 128], f32, tag='ex')
                nc.scalar.activation(ex[:], oaps[:, 128:256], AF.Exp)
                zs = wk.tile([128, 8], f32, tag='zs')
                nc.vector.tensor_reduce(zs[:], ex[:].rearrange("p (h k) -> p h k", k=16),
                                        mybir.AxisListType.X, OP.add)
                zr = wk.tile([128, 8], f32, tag='zr')
                nc.vector.reciprocal(zr[:], zs[:])
                wn = wk.tile([128, 128], f32, tag='wn')
                nc.vector.tensor_tensor(
                    wn[:].rearrange("p (h k) -> p h k", k=16),
                    ex[:].rearrange("p (h k) -> p h k", k=16),
                    zr[:].to_broadcast([128, 8, 16]), OP.mult)

                # positions: ar = ref*T - 0.5 ; fl16 = floor(ar) + 16 ; win rows
                ar = wk.tile([128, L], f32, tag='ar')
                nc.vector.tensor_tensor(ar[:], reft[:], tscS[:, :], OP.mult)
                nc.vector.tensor_scalar(ar[:], ar[:], 0.5, None, OP.subtract)
                t16 = wk.tile([128, L], f32, tag='t16')
                nc.vector.tensor_scalar(t16[:], ar[:], 16.0, None, OP.add)
                fli = wk.tile([128, L], i32, tag='fli')
                nc.vector.tensor_copy(fli[:], t16[:])
                fl16 = wk.tile([128, L], f32, tag='fl16')
                nc.vector.tensor_copy(fl16[:], fli[:])
                wrf = wk.tile([128, L], f32, tag='wrf')
                nc.vector.tensor_tensor(wrf[:], fl16[:], slcS[:, :], OP.add)
                wri = wk.tile([128, L], i32, tag='wri')
                nc.vector.tensor_copy(wri[:], wrf[:])

                # arfl = ar + 18 - fl16, expanded over points: [128, 16] (l, p)
                arf = wk.tile([128, L], f32, tag='arf')
                nc.vector.scalar_tensor_tensor(arf[:], ar[:], 18.0, fl16[:], OP.add, OP.subtract)
                arlp = wk.tile([128, 16], f32, tag='arlp')
                nc.vector.tensor_copy(arlp[:].rearrange("p (l k) -> p l k", k=NP),
                                      _bc(arf[:, :], [[1, L], [0, NP]]))
                # x = (off_nobias + b_off) + (ar + 18 - floor(ar) - 16)
                Aq = wk.tile([128, HLP], f32, tag='Aq')
                nc.vector.tensor_tensor(Aq[:], oaps[:, 0:128], boffrS[:, :], OP.add)
                xq = wk.tile([128, HLP], f32, tag='xq')
                nc.vector.tensor_tensor(
                    xq[:], Aq[:], _bc(arlp[:, :], [[0, H], [1, 16]]), OP.add)

                # hat weights: c3p = relu(1 - |riota - x|) * wn
                dd = wk.tile([128, HLP * W], f32, tag='dd', bufs=2)
                nc.vector.tensor_tensor(
                    dd[:], riotaS[:, :],
                    xq[:, :].to_broadcast([128, HLP, W]), OP.subtract)
                da = wk.tile([128, HLP * W], f32, tag='da', bufs=2)
                nc.scalar.activation(da[:], dd[:], AF.Abs)
                h1 = wk.tile([128, HLP * W], f32, tag='h1', bufs=2)
                nc.scalar.activation(h1[:], da[:], AF.Relu, bias=1.0, scale=-1.0)
                c3p = wk.tile([128, HLP * W], bf16, tag='c3p')
                nc.vector.tensor_tensor(
                    c3p[:], h1[:],
                    wn[:, :].to_broadcast([128, HLP, W]), OP.mult)

                # sum over points: c3p cols = h*96 + l*24 + p*6 + r -> c3 cols h*24+l*6+r
                c3a = wk.tile([128, H * L * W], bf16, tag='c3a')
                nc.vector.tensor_tensor(
                    c3a[:],
                    _bc(c3p[:, :], [[24, H * L], [1, W]]),
                    _bc(dataclasses.replace(c3p[:, :], offset=c3p[:, :].offset + 6),
                        [[24, H * L], [1, W]]), OP.add)
                c3b = wk.tile([128, H * L * W], bf16, tag='c3b')
                nc.vector.tensor_tensor(
                    c3b[:],
                    _bc(dataclasses.replace(c3p[:, :], offset=c3p[:, :].offset + 12),
                        [[24, H * L], [1, W]]),
                    _bc(dataclasses.replace(c3p[:, :], offset=c3p[:, :].offset + 18),
                        [[24, H * L], [1, W]]), OP.add)
                c3 = wk.tile([128, H * L * W], bf16, tag='c3')
                nc.vector.tensor_tensor(c3[:], c3a[:], c3b[:], OP.add)

                # ---- phase C: per-level gather + weighted reduce ----
                gws = []
                for lv in range(L):
                    gw = gat.tile([128, WIN], bf16, tag='gw')
                    nc.gpsimd.indirect_dma_start(
                        out=gw[:], out_offset=None, in_=vwin,
                        in_offset=bass.IndirectOffsetOnAxis(ap=wri[:, lv:lv + 1], axis=0))
                    gws.append(gw)
                wts = []
                for lv in range(L):
                    gw = gws[lv]
                    # T_r = G_r * c3[:, (h, lv, r)] broadcast over the 32 head dims
                    tt = gat.tile([128, WIN], bf16, tag='tt')
                    nc.vector.tensor_tensor(
                        tt[:].rearrange("p (r h d) -> p r h d", r=W, h=H),
                        gw[:].rearrange("p (r h d) -> p r h d", r=W, h=H),
                        _bc(dataclasses.replace(c3[:, :], offset=c3[:, :].offset + lv * W),
                            [[1, W], [24, H], [0, 32]]), OP.mult)
                    uu = wk.tile([128, 3 * D], bf16, tag='uu')
                    nc.vector.tensor_tensor(
                        uu[:].rearrange("p (a c) -> p a c", c=D),
                        _bc(tt[:, :], [[2 * D, 3], [1, D]]),
                        _bc(dataclasses.replace(tt[:, :], offset=tt[:, :].offset + D),
                            [[2 * D, 3], [1, D]]), OP.add)
                    v1 = wk.tile([128, D], bf16, tag='v1', bufs=4)
                    nc.vector.tensor_tensor(v1[:], uu[:, 0:D], uu[:, D:2 * D], OP.add)
                    wt = wk.tile([128, D], bf16, tag='wt', bufs=4)
                    nc.vector.tensor_tensor(wt[:], v1[:], uu[:, 2 * D:3 * D], OP.add)
                    wts.append(wt)
                a01 = wk.tile([128, D], bf16, tag='a01')
                nc.vector.tensor_tensor(a01[:], wts[0][:], wts[1][:], OP.add)
                a23 = wk.tile([128, D], bf16, tag='a23')
                nc.vector.tensor_tensor(a23[:], wts[2][:], wts[3][:], OP.add)
                att = wk.tile([128, D], f32, tag='att')
                nc.vector.tensor_tensor(att[:], a01[:], a23[:], OP.add)

                # ---- phase D: out-projection, LN1, FFN, LN2 ----
                attb = wk.tile([128, D], bf16, tag='attb')
                nc.scalar.activation(attb[:], att[:], AF.Copy)
                attT = wk.tile([128, D], bf16, tag='attT')
                for kc in range(2):
                    tp = ptr.tile([128, 128], bf16, tag='tr')
                    nc.tensor.transpose(tp[:], attb[:, kc * 128:(kc + 1) * 128], identS[:])
                    nc.scalar.activation(attT[:, kc * 128:(kc + 1) * 128], tp[:], AF.Copy)
                s2ps = p256.tile([128, D], f32, tag='p256')
                nc.tensor.matmul(s2ps[:], attT[:, 0:128], woutS[:, 0:D], start=True, stop=False)
                nc.tensor.matmul(s2ps[:], attT[:, 128:256], woutS[:, D:2 * D], start=False, stop=False)
                nc.tensor.matmul(s2ps[:], onesS[:1, :], boutS[:1, :], start=False, stop=True)

                # LN1 + time_embed
                sfull = wk.tile([128, D], f32, tag='sfull')
                nc.vector.tensor_tensor(sfull[:], srco[:], s2ps[:], OP.add)
                ssum = wk.tile([128, 1], f32, tag='ssum')
                nc.vector.tensor_reduce(ssum[:], sfull[:], mybir.AxisListType.X, OP.add)
                mu = wk.tile([128, 1], f32, tag='mu')
                nc.vector.tensor_scalar(mu[:], ssum[:], 1.0 / D, None, OP.mult)
                xc = wk.tile([128, D], f32, tag='xc')
                nc.vector.tensor_scalar(xc[:], sfull[:], mu[:, 0:1], None, OP.subtract)
                sq = wk.tile([128, D], f32, tag='sq')
                var = wk.tile([128, 1], f32, tag='var')
                nc.vector.scalar_tensor_tensor(sq[:], xc[:], 0.0, xc[:], OP.add, OP.mult,
                                               accum_out=var[:])
                sd = wk.tile([128, 1], f32, tag='sd')
                nc.scalar.activation(sd[:], var[:], AF.Sqrt, bias=epsS[:, 0:1], scale=1.0 / D)
                rsd = wk.tile([128, 1], f32, tag='rsd')
                nc.vector.reciprocal(rsd[:], sd[:])
                xn = wk.tile([128, D], f32, tag='xn')
                nc.scalar.activation(xn[:], xc[:], AF.Copy, scale=rsd[:, 0:1])
                teb = wk.tile([128, D], f32, tag='teb')
                nc.gpsimd.tensor_tensor(teb[:], tet[:], ln1bS[:, :], OP.add)
                t1 = wk.tile([128, D], f32, tag='t1')
                nc.gpsimd.tensor_tensor(t1[:], xn[:], ln1gS[:, :], OP.mult)
                xx = wk.tile([128, D], f32, tag='xx')
                nc.gpsimd.tensor_tensor(xx[:], t1[:], teb[:], OP.add)

                # FFN
                xb = wk.tile([128, D], bf16, tag='xb')
                nc.scalar.activation(xb[:], xx[:], AF.Copy)
                xT = wk.tile([128, D], bf16, tag='xT')
                for kc in range(2):
                    tp = ptr.tile([128, 128], bf16, tag='tr')
                    nc.tensor.transpose(tp[:], xb[:, kc * 128:(kc + 1) * 128], identS[:])
                    nc.scalar.activation(xT[:, kc * 128:(kc + 1) * 128], tp[:], AF.Copy)
                hbT = wk.tile([128, DF], bf16, tag='hbT')
                for k in range(8):
                    hps = ptr.tile([128, 128], f32, tag='tr')
                    for kc in range(2):
                        nc.tensor.matmul(hps[:], w1S[:, kc * DF + k * 128: kc * DF + (k + 1) * 128],
                                         xT[:, kc * 128:(kc + 1) * 128],
                                         start=(kc == 0), stop=(kc == 1))
                    nc.scalar.activation(hbT[:, k * 128:(k + 1) * 128], hps[:], AF.Relu,
                                         bias=b1tS[:, k:k + 1])
                o2ps = p256.tile([128, D], f32, tag='p256')
                for k in range(8):
                    nc.tensor.matmul(o2ps[:], hbT[:, k * 128:(k + 1) * 128],
                                     w2S[:, k * D:(k + 1) * D],
                                     start=(k == 0), stop=False)
                nc.tensor.matmul(o2ps[:], onesS[:1, :], b2S[:1, :], start=False, stop=True)

                # LN2
                sf2 = wk.tile([128, D], f32, tag='sf2')
                nc.vector.tensor_tensor(sf2[:], xx[:], o2ps[:], OP.add)
                ssum2 = wk.tile([128, 1], f32, tag='ssum2')
                nc.vector.tensor_reduce(ssum2[:], sf2[:], mybir.AxisListType.X, OP.add)
                mu2 = wk.tile([128, 1], f32, tag='mu2')
                nc.vector.tensor_scalar(mu2[:], ssum2[:], 1.0 / D, None, OP.mult)
                xc2 = wk.tile([128, D], f32, tag='xc2')
                nc.vector.tensor_scalar(xc2[:], sf2[:], mu2[:, 0:1], None, OP.subtract)
                sq2 = wk.tile([128, D], f32, tag='sq2')
                var2 = wk.tile([128, 1], f32, tag='var2')
                nc.vector.scalar_tensor_tensor(sq2[:], xc2[:], 0.0, xc2[:], OP.add, OP.mult,
                                               accum_out=var2[:])
                sd2 = wk.tile([128, 1], f32, tag='sd2')
                nc.scalar.activation(sd2[:], var2[:], AF.Sqrt, bias=epsS[:, 0:1], scale=1.0 / D)
                rsd2 = wk.tile([128, 1], f32, tag='rsd2')
                nc.vector.reciprocal(rsd2[:], sd2[:])
                xn2 = wk.tile([128, D], f32, tag='xn2')
                nc.scalar.activation(xn2[:], xc2[:], AF.Copy, scale=rsd2[:, 0:1])
                t2 = wk.tile([128, D], f32, tag='t2')
                nc.gpsimd.tensor_tensor(t2[:], xn2[:], ln2gS[:, :], OP.mult)
                of = wk.tile([128, D], f32, tag='of')
                nc.gpsimd.tensor_tensor(of[:], t2[:], ln2bS[:, :], OP.add)
                nc.sync.dma_start(out_q[rq:rq + 128, :], of[:])

    if os.environ.get('K_NOSPLIT', '0') != '1':
        _split_excess_waits(nc)
    return nc


_PROG = None
LAST_RESULTS = None


def _get_program():
    global _PROG
    if _PROG is None:
        _PROG = _build_program()
    return _PROG


def _host_consts():
    bfc = lambda a: np.ascontiguousarray(a).astype(ml_dtypes.bfloat16)
    c = {}
    c['ident'] = bfc(np.eye(128, dtype=np.float32))
    c['ones1'] = bfc(np.ones((1, 128), np.float32))
    em = np.zeros((H * W, WIN), np.float32)
    for h in range(H):
        for r in range(W):
            em[h * W + r, r * D + h * 32:(r * D) + (h + 1) * 32] = 1.0
    c['emat'] = bfc(em)
    ri = np.tile(np.arange(W, dtype=np.float32), HLP)[None, :]
    c['riota'] = np.repeat(ri, 128, axis=0).astype(np.float32)
    c['tsc'] = np.repeat(np.array([LEVEL_LENS], np.float32), 128, axis=0)
    c['slc'] = np.repeat(np.array([[PSTARTS[lv] - 18 for lv in range(L)]], np.float32),
                         128, axis=0)
    return c


def kernel(src, pos, time_embed, reference_points, w_off, b_off, w_attn, b_attn,
           w_val, b_val, w_out, b_out, ln1_g, ln1_b, w1, b1, w2, b2, ln2_g, ln2_b,
           spatial_shapes, level_start_index):
    src = np.asarray(src, np.float32)
    pos = np.asarray(pos, np.float32)
    te = np.asarray(time_embed, np.float32)
    ref = np.asarray(reference_points, np.float32).reshape(2, LEN_IN, L)

    bfc = lambda a: np.ascontiguousarray(np.asarray(a, np.float32)).astype(ml_dtypes.bfloat16)
    consts = _host_consts()

    woa_full = np.concatenate([np.asarray(w_off, np.float32),
                               np.asarray(w_attn, np.float32)], axis=1)  # [256, 256]
    boa_full = np.concatenate([np.asarray(b_off, np.float32),
                               np.asarray(b_attn, np.float32)])[None, :]

    def chunk2(w):  # [256, X] -> [128, 2X]
        w = np.asarray(w, np.float32)
        return np.concatenate([w[0:128, :], w[128:256, :]], axis=1)

    w2f = np.asarray(w2, np.float32)
    w2c = np.concatenate([w2f[k * 128:(k + 1) * 128, :] for k in range(8)], axis=1)
    b1f = np.asarray(b1, np.float32)
    b1t = np.stack([b1f[k * 128:(k + 1) * 128] for k in range(8)], axis=1)  # [128, 8]
    rep = lambda v: np.repeat(np.asarray(v, np.float32)[None, :], 128, axis=0)

    shared = {
        'wval': chunk2(bfc(w_val).astype(np.float32)) .astype(ml_dtypes.bfloat16),
        'bval': bfc(np.asarray(b_val, np.float32)[None, :]),
        'woa': bfc(chunk2(woa_full)),
        'boa': bfc(boa_full),
        'wout': bfc(chunk2(np.asarray(w_out, np.float32))),
        'bout': bfc(np.asarray(b_out, np.float32)[None, :]),
        'w1': bfc(chunk2(np.asarray(w1, np.float32))),
        'b1t': b1t.astype(np.float32),
        'w2': bfc(w2c),
        'b2': bfc(np.asarray(b2, np.float32)[None, :]),
        'boffr': np.repeat(np.asarray(b_off, np.float32)[None, :], 128, axis=0),
        'ln1g': rep(ln1_g), 'ln1b': rep(ln1_b),
        'ln2g': rep(ln2_g), 'ln2b': rep(ln2_b),
        **consts,
    }
    # fix wval double-cast
    shared['wval'] = bfc(chunk2(np.asarray(w_val, np.float32)))

    in_maps = []
    for c in range(N_CORES):
        n, qr = c // 4, c % 4
        m = dict(shared)
        m['src_full'] = src[n]
        m['srcq'] = src[n, qr * QC:(qr + 1) * QC]
        m['pos_q'] = pos[n, qr * QC:(qr + 1) * QC]
        m['te_q'] = te[n, qr * QC:(qr + 1) * QC]
        m['ref_q'] = ref[n, qr * QC:(qr + 1) * QC]
        in_maps.append(m)

    nc = _get_program()
    from concourse.bass_utils import run_bass_kernel_spmd
    res = run_bass_kernel_spmd(nc, in_maps, core_ids=list(range(N_CORES)))
    global LAST_RESULTS
    LAST_RESULTS = res
    if getattr(res, 'exec_time_ns', None):
        print('HW exec time:', res.exec_time_ns, 'ns')

    out = np.zeros((2, LEN_IN, D), np.float32)
    for c in range(N_CORES):
        n, qr = c // 4, c % 4
        out[n, qr * QC:(qr + 1) * QC] = res.results[c]['out_q']
    return out



# revision 29
# speedup vs baseline: 1.2252x; 1.2252x over previous
"""Deformable transformer encoder layer on 8 Trainium2 NeuronCores (v2).

Sharding: core c handles batch c//4, query-quarter c%4 (3840 queries each).

v2 design (vs v1 baseline):
  - fp16 activations/weights throughout (DVE 2x modes, halved gather bytes).
  - value table columns permuted to (k2, h, u) so the per-level window
    multiply runs with a 3-4-dim AP whose innermost dims are stride-1 on
    both operands -> DVE 2x_1p mode.
  - one SWDGE dma_gather per 384-query macro-tile fetches all 4 levels'
    6-row windows (1536 indices, host-precomputed int16, SWDGE fixed
    overhead amortized 12x vs per-(tile,level) indirect DMAs).
  - hat weights relu(1-|x-r|) via ScalarE Abs/Relu with immediate biases;
    rsqrt(var) = Exp(-0.5*Ln(var+eps)) so every ScalarE function used
    ({Exp, Abs, Relu, Copy, Ln}) lives in one act table -> no reloads.
  - residuals (src +, x +) folded into the out-proj / FFN-w2 matmuls as
    identity-matrix matmuls accumulating in PSUM (no DVE residual adds,
    src never loaded untransposed).
  - all direct DMAs issued from gpsimd (25ns issue vs 565ns on sync).
"""
import os
import sys

sys.path.insert(0, '/opt/trn_rl_repo')

import dataclasses
import numpy as np
import ml_dtypes

import concourse.bass as bass
import concourse.mybir as mybir
from concourse.tile import TileContext
from concourse import library_config
from concourse.library_overlay import lower_extended_insts

# ---- tile drain workaround (this walrus rejects multi-wait drains) ----
import concourse.tile as _tile_mod
from concourse.tile_sem_assignment import tick_to_sem as _tick_to_sem


def _split_drain_and_barrier(self, tick_clock, wait_clock):
    gc = tick_clock.global_clock
    allocated = self.sems.allocated() if self.sems is not None else {}
    for proc, sem in sorted(allocated.items()):
        t = gc[proc]
        if t > 0:
            self.nc.sync.wait_ge(sem, _tick_to_sem(t, proc))
    self.nc.sync.drain()
    self.nc.all_engine_barrier()
    assert self.sems is not None
    popped = self.nc._tile_sem_poison_stack.pop()
    assert popped is self._sem_poison
    self.nc.clear_and_free_semaphores(list(self.sems.allocated().values()))
    self.nc.all_engine_barrier()


_tile_mod.TileContext._drain_and_barrier = _split_drain_and_barrier

_MAX_WAITS = 1
_wsplit_n = [0]


def _split_excess_waits(nc):
    """Walrus rejects instructions with >2 sem waits; move extras to nops."""
    for f in nc.m.functions:
        for bb in f.blocks:
            out = []
            for inst in list(bb.instructions):
                si = inst.sync_info
                waits = list(si.on_wait) if (si and si.on_wait) else []
                if len(waits) > _MAX_WAITS:
                    extra = waits[:-_MAX_WAITS]
                    keep = waits[-_MAX_WAITS:]
                    for j in range(0, len(extra), _MAX_WAITS):
                        _wsplit_n[0] += 1
                        nop = mybir.InstNoOp(name=f'wsplit-{_wsplit_n[0]}',
                                             ins=[], outs=[])
                        nop.engine = inst.engine
                        nop.sync_info = mybir.SyncInfo(
                            on_wait=extra[j:j + _MAX_WAITS], on_update=[])
                        out.append(nop)
                    inst.sync_info = mybir.SyncInfo(
                        on_wait=keep, on_update=list(si.on_update or []))
                out.append(inst)
            bb.instructions = out


f32 = mybir.dt.float32
fp16 = mybir.dt.float16
i16 = mybir.dt.int16
AF = mybir.ActivationFunctionType
OP = mybir.AluOpType

# ---- problem constants ----
D = 256
DF = 1024
H = 8
L = 4
NP = 4
LEVEL_LENS = (8192, 4096, 2048, 1024)
LEN_IN = 15360
N_CORES = 8
EPS = 1e-5

W = 6
PAD = 8
_starts = []
_acc = PAD
for _t in LEVEL_LENS:
    _starts.append(_acc)
    _acc += _t + PAD
PSTARTS = tuple(_starts)
VROWS = _acc               # 15400
QC = LEN_IN // 4           # 3840
NT_V = LEN_IN // 128       # 120
MACRO = 384                # queries per macro-tile
JT = MACRO // 128          # 3
NMAC = QC // MACRO         # 10
WIN = W * D                # 1536
NIDX = MACRO * L           # 1536 gather indices per macro
CUMS = [0]
for _t in LEVEL_LENS:
    CUMS.append(CUMS[-1] + _t)


def _bc(ap, dims, extra_off=0):
    """Replace the free dims of an AP with an explicit dim list."""
    ap2 = dataclasses.replace(
        ap, ap=[list(ap.ap[0])] + [list(d) for d in dims])
    if extra_off:
        ap2 = dataclasses.replace(ap2, offset=ap2.offset + extra_off)
    return ap2


def _dap(ap, dims, extra_off=0):
    """Replace the WHOLE AP dim list (incl. leading/partition dim)."""
    ap2 = dataclasses.replace(ap, ap=[list(d) for d in dims])
    if extra_off:
        ap2 = dataclasses.replace(ap2, offset=ap2.offset + extra_off)
    return ap2


def _build_program():
    nc = bass.Bass(trn_type='TRN2')

    din = {}

    def I(name, shape, dt):
        din[name] = nc.dram_tensor(name, shape, dt, kind='ExternalInput')
        return din[name]

    srcTk = I('srcTk', [128, 2, LEN_IN], fp16)   # src^T, k-chunked (full batch)
    srcTq = I('srcTq', [128, 2, QC], fp16)       # src^T (this core's quarter)
    posTk = I('posTk', [128, 2, QC], fp16)       # pos^T (quarter)
    teb = I('teb', [QC, D], fp16)                # time_embed + ln1_b
    arfq = I('arfq', [QC, L], f32)               # frac(ar) + 2
    widx = I('widx', [128, NMAC * (NIDX // 16)], i16)

    ident = I('ident', [128, 128], fp16)
    ones1 = I('ones1', [1, 128], fp16)
    ires = I('ires', [128, 2, D], fp16)          # identity for PSUM residual
    wvalP = I('wvalP', [128, 2, D], fp16)
    bvalP = I('bvalP', [1, D], fp16)
    woa = I('woa', [128, 2, D], fp16)
    boa = I('boa', [1, D], fp16)
    woutR = I('woutR', [128, 2, D], fp16)
    bout = I('bout', [1, D], fp16)
    w1 = I('w1', [128, 2, DF], fp16)
    b1t = I('b1t', [128, 8], f32)
    w2 = I('w2', [128, 8, D], fp16)
    b2 = I('b2', [1, D], fp16)
    ln1g = I('ln1g', [128, D], fp16)
    ln2g = I('ln2g', [128, D], fp16)
    ln2b = I('ln2b', [128, D], fp16)
    cbias = I('cbias', [128, 8], f32)   # cols 0..5: -r ; col 6: eps

    out_q = nc.dram_tensor('out_q', [QC, D], fp16, kind='ExternalOutput')

    with TileContext(nc) as tc:
        with tc.tile_pool(name='cst', bufs=1) as cst, \
             tc.tile_pool(name='io', bufs=3) as io, \
             tc.tile_pool(name='wk', bufs=1) as wk, \
             tc.tile_pool(name='gat', bufs=2) as gat, \
             tc.tile_pool(name='p256', bufs=3, space='PSUM') as p256, \
             tc.tile_pool(name='ptr', bufs=2, space='PSUM') as ptr, \
             tc.tile_pool(name='php', bufs=2, space='PSUM') as php, \
             tc.tile_pool(name='dram', bufs=1, space='DRAM') as dram:

            nc.gpsimd.load_library(library_config.mlp)

            vtab = dram.tile([VROWS, D], fp16)

            def ctile(name, dt=fp16):
                t = cst.tile(list(din[name].shape), dt, tag=name)
                nc.gpsimd.dma_start(
                    t[:], din[name][tuple(slice(None) for _ in din[name].shape)])
                return t

            identS = ctile('ident')
            ones1S = ctile('ones1')
            iresS = ctile('ires')
            wvalS = ctile('wvalP')
            bvalS = ctile('bvalP')
            woaS = ctile('woa')
            boaS = ctile('boa')
            woutS = ctile('woutR')
            boutS = ctile('bout')
            w1S = ctile('w1')
            b1tS = ctile('b1t', f32)
            w2S = ctile('w2')
            b2S = ctile('b2')
            ln1gS = ctile('ln1g')
            ln2gS = ctile('ln2g')
            ln2bS = ctile('ln2b')
            cbiasS = ctile('cbias', f32)

            # ---- zero pad rows of value table ----
            zpad = cst.tile([PAD, D], fp16, tag='zpad')
            nc.vector.memset(zpad[:], 0.0)
            nc.gpsimd.dma_start(vtab[0:PAD, :], zpad[:])
            for lv in range(L):
                r0 = PSTARTS[lv] + LEVEL_LENS[lv]
                nc.gpsimd.dma_start(vtab[r0:r0 + PAD, :], zpad[:])

            # ---- phase A: value projection ----
            for i in range(int(os.environ.get('K_NTV', NT_V))):
                r = i * 128
                lv = next(k for k in range(L) if r < CUMS[k + 1])
                prow = PSTARTS[lv] + (r - CUMS[lv])

                st = io.tile([128, 2, 128], fp16, tag='va_in')
                nc.gpsimd.dma_start(st[:], srcTk[:, :, r:r + 128])
                vps = p256.tile([128, D], f32, tag='p256')
                nc.tensor.matmul(vps[:], st[:, 0, :], wvalS[:, 0, :],
                                 start=True, stop=False)
                nc.tensor.matmul(vps[:], st[:, 1, :], wvalS[:, 1, :],
                                 start=False, stop=False)
                nc.tensor.matmul(vps[:], ones1S[:1, :], bvalS[:1, :],
                                 start=False, stop=True)
                vb = wk.tile([128, D], fp16, tag='va_out', bufs=3)
                nc.scalar.activation(vb[:], vps[:], AF.Copy)
                nc.gpsimd.dma_start(vtab[prow:prow + 128, :], vb[:])

            vwin = dataclasses.replace(
                vtab[:, :], ap=[[D, VROWS - W], [1, WIN]])

            # ---- phase B: macro-tiles ----
            for m in range(int(os.environ.get('K_NMAC', NMAC))):
                q0 = m * MACRO

                sTq = io.tile([128, 2, MACRO], fp16, tag='sTq')
                nc.gpsimd.dma_start(sTq[:], srcTq[:, :, q0:q0 + MACRO])
                pTq = io.tile([128, 2, MACRO], fp16, tag='pTq')
                nc.gpsimd.dma_start(pTq[:], posTk[:, :, q0:q0 + MACRO])
                tebS = io.tile([128, JT, D], fp16, tag='tebS')
                nc.gpsimd.dma_start(
                    tebS[:],
                    _dap(teb[:, :], [[D, 128], [128 * D, JT], [1, D]],
                         extra_off=q0 * D))
                arfS = io.tile([128, JT, L], f32, tag='arfS')
                nc.gpsimd.dma_start(
                    arfS[:],
                    _dap(arfq[:, :], [[L, 128], [128 * L, JT], [1, L]],
                         extra_off=q0 * L))
                wxS = io.tile([128, NIDX // 16], i16, tag='wxS')
                nc.gpsimd.dma_start(
                    wxS[:], widx[:, m * (NIDX // 16):(m + 1) * (NIDX // 16)])

                # gather: all 4 levels x 3 subtiles, 6-row windows.
                # two calls of 768 idxs (SWDGE ring caps out between 1024
                # and 1536 descriptors per call)
                gw = gat.tile([128, L * JT, WIN], fp16, tag='gw')
                if os.environ.get('K_NOGATHER', '0') == '1':
                    nc.vector.memset(gw[:], 0.01)
                else:
                    half = NIDX // 2
                    for g2 in range(2):
                        nc.gpsimd.dma_gather(
                            out_ap=gw[:, g2 * (L * JT // 2):(g2 + 1) * (L * JT // 2), :],
                            in_ap=vwin,
                            idxs_ap=wxS[:, g2 * (half // 16):(g2 + 1) * (half // 16)],
                            num_idxs=half, num_idxs_reg=half,
                            elem_size=WIN, elem_step=D)

                # qT = (src + pos)^T
                qT = wk.tile([128, 2, MACRO], fp16, tag='qT', bufs=2)
                nc.vector.tensor_tensor(qT[:], sTq[:], pTq[:], OP.add)

                # off/attn projection per subtile
                oapss = []
                for j in range(JT):
                    oaps = p256.tile([128, D], f32, tag='p256')
                    nc.tensor.matmul(oaps[:], qT[:, 0, j * 128:(j + 1) * 128],
                                     woaS[:, 0, :], start=True, stop=False)
                    nc.tensor.matmul(oaps[:], qT[:, 1, j * 128:(j + 1) * 128],
                                     woaS[:, 1, :], start=False, stop=False)
                    nc.tensor.matmul(oaps[:], ones1S[:1, :], boaS[:1, :],
                                     start=False, stop=True)
                    oapss.append(oaps)

                # softmax over k=(l,p) per (j, h); attn cols are (l, p, h)
                exS = wk.tile([128, JT * 128], fp16, tag='exS')
                for j in range(JT):
                    nc.scalar.activation(exS[:, j * 128:(j + 1) * 128],
                                         oapss[j][:, 128:256], AF.Exp)
                zs = wk.tile([128, JT * H], f32, tag='zs')
                nc.vector.tensor_reduce(
                    zs[:],
                    _bc(exS[:, :], [[128, JT], [1, H], [8, 16]]),
                    mybir.AxisListType.X, OP.add)
                zr = wk.tile([128, JT * H], fp16, tag='zr')
                with nc.allow_low_precision(reason='softmax denom fp16 is ample'):
                    nc.vector.reciprocal(zr[:], zs[:])
                wn = wk.tile([128, JT * 128], fp16, tag='wn')
                nc.vector.tensor_tensor(
                    _bc(wn[:, :], [[128, JT], [8, 16], [1, H]]),
                    _bc(exS[:, :], [[128, JT], [8, 16], [1, H]]),
                    _bc(zr[:, :], [[H, JT], [0, 16], [1, H]]), OP.mult)

                # xq[p, (j,l,p4,h)] = off + arf  (off cols are (l, p4, h))
                xq = wk.tile([128, JT * 128], fp16, tag='xq')
                for j in range(JT):
                    nc.vector.tensor_tensor(
                        xq[:, j * 128:(j + 1) * 128],
                        oapss[j][:, 0:128],
                        _bc(arfS[:, :, :], [[1, L], [0, NP * H]],
                            extra_off=j * L), OP.add)

                # hat: h1[r] = relu(1 - |xq - r|)
                da = wk.tile([128, W * JT * 128], fp16, tag='da')
                for r in range(W):
                    nc.scalar.activation(
                        da[:, r * JT * 128:(r + 1) * JT * 128], xq[:],
                        AF.Abs, bias=cbiasS[:, r:r + 1], scale=1.0)
                h1 = wk.tile([128, W * JT * 128], fp16, tag='h1')
                for r in range(W):
                    nc.scalar.activation(
                        h1[:, r * JT * 128:(r + 1) * JT * 128],
                        da[:, r * JT * 128:(r + 1) * JT * 128],
                        AF.Relu, bias=1.0, scale=-1.0)

                # c3p[(j,l,r,p4,h)] = h1[(r,j,l,p4,h)] * wn[(j,l,p4,h)]
                c3p = wk.tile([128, W * JT * 128], fp16, tag='c3p')
                nc.vector.tensor_tensor(
                    _bc(c3p[:, :], [[192, JT * L], [32, W], [1, NP * H]]),
                    _bc(h1[:, :], [[32, JT * L], [384, W], [1, NP * H]]),
                    _bc(wn[:, :], [[32, JT * L], [0, W], [1, NP * H]]),
                    OP.mult)

                # fold sampling points: A[(j,l,r), u, h] = p_u + p_{u+2}
                Af = wk.tile([128, W * JT * 64], fp16, tag='Af')
                nc.vector.tensor_tensor(
                    Af[:],
                    _bc(c3p[:, :], [[32, JT * L * W], [8, 2], [1, H]]),
                    _bc(c3p[:, :], [[32, JT * L * W], [8, 2], [1, H]],
                        extra_off=16), OP.add)
                # c3d[(j,l,r), h, u'] = A[..., 0, h] + A[..., 1, h], dup x2
                c3d = wk.tile([128, JT * 384], fp16, tag='c3d')
                for up in range(2):
                    nc.vector.tensor_tensor(
                        _bc(c3d[:, :], [[16, JT * L * W], [2, H]],
                            extra_off=up),
                        _bc(Af[:, :], [[16, JT * L * W], [1, H]]),
                        _bc(Af[:, :], [[16, JT * L * W], [1, H]],
                            extra_off=8), OP.add)

                # sampling reduce per subtile
                attjs = []
                for j in range(JT):
                    tt = gat.tile([128, L * W * D], fp16, tag='tt', bufs=1)
                    nc.vector.tensor_tensor(
                        tt[:],
                        _bc(gw[:, :, :], [[JT * WIN, L], [D, W], [1, D]],
                            extra_off=j * WIN),
                        _bc(c3d[:, :], [[16, L * W], [0, 16], [1, 16]],
                            extra_off=j * 384), OP.mult)
                    Y = wk.tile([128, L * 3 * D], fp16, tag='Yf')
                    nc.vector.tensor_tensor(
                        Y[:],
                        _bc(tt[:, :], [[6 * D, L], [D, 3], [1, D]]),
                        _bc(tt[:, :], [[6 * D, L], [D, 3], [1, D]],
                            extra_off=3 * D), OP.add)
                    Zt = wk.tile([128, L * D], fp16, tag='Zf')
                    nc.vector.tensor_tensor(
                        Zt[:],
                        _bc(Y[:, :], [[3 * D, L], [1, D]]),
                        _bc(Y[:, :], [[3 * D, L], [1, D]], extra_off=D),
                        OP.add)
                    at0 = wk.tile([128, L * D], fp16, tag='at0')
                    nc.vector.tensor_tensor(
                        at0[:], Zt[:],
                        _bc(Y[:, :], [[3 * D, L], [1, D]], extra_off=2 * D),
                        OP.add)
                    s01 = wk.tile([128, 2 * D], fp16, tag='s01')
                    nc.vector.tensor_tensor(s01[:], at0[:, 0:2 * D],
                                            at0[:, 2 * D:4 * D], OP.add)
                    attj = wk.tile([128, D], fp16, tag='attj', bufs=3)
                    nc.vector.tensor_tensor(attj[:], s01[:, 0:D],
                                            s01[:, D:2 * D], OP.add)
                    attjs.append(attj)

                # out-projection + src residual (PSUM) + LN1
                xx = wk.tile([128, JT * D], fp16, tag='xx')
                xT = wk.tile([128, 2, MACRO], fp16, tag='xT')
                for j in range(JT):
                    attT = wk.tile([128, 2, 128], fp16, tag='attT')
                    for c in range(2):
                        tp = ptr.tile([128, 128], fp16, tag='tr')
                        nc.tensor.transpose(
                            tp[:], attjs[j][:, c * 128:(c + 1) * 128],
                            identS[:])
                        nc.scalar.activation(attT[:, c, :], tp[:], AF.Copy)
                    s2ps = p256.tile([128, D], f32, tag='p256')
                    nc.tensor.matmul(s2ps[:], attT[:, 0, :], woutS[:, 0, :],
                                     start=True, stop=False)
                    nc.tensor.matmul(s2ps[:], attT[:, 1, :], woutS[:, 1, :],
                                     start=False, stop=False)
                    for c in range(2):
                        nc.tensor.matmul(
                            s2ps[:],
                            sTq[:, c, j * 128:(j + 1) * 128],
                            iresS[:, c, :], start=False, stop=False)
                    nc.tensor.matmul(s2ps[:], ones1S[:1, :], boutS[:1, :],
                                     start=False, stop=True)

                    # LN1 on s2ps: stats via ScalarE accumulators
                    s2f = wk.tile([128, D], fp16, tag='s2f')
                    ms = wk.tile([128, 1], f32, tag='ms')
                    nc.scalar.activation(s2f[:], s2ps[:], AF.Copy,
                                         accum_out=ms[:])
                    sq = wk.tile([128, D], fp16, tag='sq')
                    ssq = wk.tile([128, 1], f32, tag='ssq')
                    nc.scalar.activation(sq[:], s2ps[:], AF.Square,
                                         accum_out=ssq[:])
                    mu = wk.tile([128, 1], f32, tag='mu')
                    nc.vector.tensor_scalar(mu[:], ms[:], 1.0 / D, None,
                                            OP.mult)
                    mu2 = wk.tile([128, 1], f32, tag='mu2')
                    nc.vector.tensor_tensor(mu2[:], mu[:], mu[:], OP.mult)
                    var0 = wk.tile([128, 1], f32, tag='var0')
                    nc.vector.scalar_tensor_tensor(
                        var0[:], ssq[:], 1.0 / D, mu2[:], OP.mult,
                        OP.subtract)
                    lnv = wk.tile([128, 1], f32, tag='lnv')
                    nc.scalar.activation(lnv[:], var0[:], AF.Ln,
                                         bias=cbiasS[:, 6:7], scale=1.0)
                    rsd = wk.tile([128, 1], f32, tag='rsd')
                    nc.scalar.activation(rsd[:], lnv[:], AF.Exp, bias=0.0,
                                         scale=-0.5)
                    xn = wk.tile([128, D], fp16, tag='xn')
                    nc.vector.tensor_scalar(xn[:], s2f[:], mu[:, 0:1],
                                            rsd[:, 0:1], OP.subtract,
                                            OP.mult)
                    t1 = wk.tile([128, D], fp16, tag='t1')
                    nc.vector.tensor_tensor(t1[:], xn[:], ln1gS[:, :],
                                            OP.mult)
                    nc.vector.tensor_tensor(xx[:, j * D:(j + 1) * D], t1[:],
                                            tebS[:, j, :], OP.add)
                    # transpose xx_j for FFN + residual
                    for c in range(2):
                        tp = ptr.tile([128, 128], fp16, tag='tr')
                        nc.tensor.transpose(
                            tp[:], xx[:, j * D + c * 128:j * D + (c + 1) * 128],
                            identS[:])
                        nc.scalar.activation(xT[:, c, j * 128:(j + 1) * 128],
                                             tp[:], AF.Copy)

                # FFN w1 (all subtiles at once)
                hbT = wk.tile([128, 8, MACRO], fp16, tag='hbT')
                for k in range(8):
                    hps = php.tile([128, MACRO], f32, tag='hps')
                    nc.tensor.matmul(hps[:], w1S[:, 0, k * 128:(k + 1) * 128],
                                     xT[:, 0, :], start=True, stop=False)
                    nc.tensor.matmul(hps[:], w1S[:, 1, k * 128:(k + 1) * 128],
                                     xT[:, 1, :], start=False, stop=True)
                    nc.scalar.activation(hbT[:, k, :], hps[:], AF.Relu,
                                         bias=b1tS[:, k:k + 1])

                # FFN w2 + xx residual + LN2
                of = wk.tile([128, JT * D], fp16, tag='of', bufs=2)
                for j in range(JT):
                    o2ps = p256.tile([128, D], f32, tag='p256')
                    for k in range(8):
                        nc.tensor.matmul(
                            o2ps[:], hbT[:, k, j * 128:(j + 1) * 128],
                            w2S[:, k, :], start=(k == 0), stop=False)
                    for c in range(2):
                        nc.tensor.matmul(
                            o2ps[:], xT[:, c, j * 128:(j + 1) * 128],
                            iresS[:, c, :], start=False, stop=False)
                    nc.tensor.matmul(o2ps[:], ones1S[:1, :], b2S[:1, :],
                                     start=False, stop=True)

                    o2f = wk.tile([128, D], fp16, tag='o2f')
                    ms2 = wk.tile([128, 1], f32, tag='ms2')
                    nc.scalar.activation(o2f[:], o2ps[:], AF.Copy,
                                         accum_out=ms2[:])
                    sq2 = wk.tile([128, D], fp16, tag='sq2')
                    ssq2 = wk.tile([128, 1], f32, tag='ssq2')
                    nc.scalar.activation(sq2[:], o2ps[:], AF.Square,
                                         accum_out=ssq2[:])
                    mu_2 = wk.tile([128, 1], f32, tag='mu_2')
                    nc.vector.tensor_scalar(mu_2[:], ms2[:], 1.0 / D, None,
                                            OP.mult)
                    mu22 = wk.tile([128, 1], f32, tag='mu22')
                    nc.vector.tensor_tensor(mu22[:], mu_2[:], mu_2[:],
                                            OP.mult)
                    var2 = wk.tile([128, 1], f32, tag='var2')
                    nc.vector.scalar_tensor_tensor(
                        var2[:], ssq2[:], 1.0 / D, mu22[:], OP.mult,
                        OP.subtract)
                    lnv2 = wk.tile([128, 1], f32, tag='lnv2')
                    nc.scalar.activation(lnv2[:], var2[:], AF.Ln,
                                         bias=cbiasS[:, 6:7], scale=1.0)
                    rsd2 = wk.tile([128, 1], f32, tag='rsd2')
                    nc.scalar.activation(rsd2[:], lnv2[:], AF.Exp, bias=0.0,
                                         scale=-0.5)
                    xn2 = wk.tile([128, D], fp16, tag='xn2')
                    nc.vector.tensor_scalar(xn2[:], o2f[:], mu_2[:, 0:1],
                                            rsd2[:, 0:1], OP.subtract,
                                            OP.mult)
                    t2 = wk.tile([128, D], fp16, tag='t2')
                    nc.vector.tensor_tensor(t2[:], xn2[:], ln2gS[:, :],
                                            OP.mult)
                    nc.vector.tensor_tensor(of[:, j * D:(j + 1) * D], t2[:],
                                            ln2bS[:, :], OP.add)

                nc.gpsimd.dma_start(
                    _dap(out_q[:, :], [[D, 128], [128 * D, JT], [1, D]],
                         extra_off=q0 * D),
                    of[:])

    if os.environ.get('K_NOSPLIT', '0') != '1':
        _split_excess_waits(nc)
    lower_extended_insts(nc)
    return nc


_PROG = None
LAST_RESULTS = None


def _get_program():
    global _PROG
    if _PROG is None:
        _PROG = _build_program()
    return _PROG


# column permutation: new col (k2, h, u) <- old col h*32 + k2*2 + u
_PERM = np.zeros(D, np.int64)
for _k2 in range(16):
    for _h in range(H):
        for _u in range(2):
            _PERM[_k2 * 16 + _h * 2 + _u] = _h * 32 + _k2 * 2 + _u

# off/attn projection column permutation: new col (l, p, h) <- old (h, l, p)
_OAPERM = np.zeros(128, np.int64)
for _l in range(L):
    for _p in range(NP):
        for _h in range(H):
            _OAPERM[_l * 32 + _p * 8 + _h] = _h * 16 + _l * 4 + _p


def _host_consts():
    c = {}
    c['ident'] = np.eye(128, dtype=np.float16)
    c['ones1'] = np.ones((1, 128), np.float16)
    ir = np.zeros((128, 2, D), np.float16)
    for p in range(128):
        ir[p, 0, p] = 1.0
        ir[p, 1, 128 + p] = 1.0
    c['ires'] = ir
    return c


def _kchunk(w):
    """[256, X] f32 -> [128, 2, X] fp16"""
    w = np.asarray(w, np.float32)
    return np.stack([w[0:128], w[128:256]], axis=1).astype(np.float16)


def kernel(src, pos, time_embed, reference_points, w_off, b_off, w_attn,
           b_attn, w_val, b_val, w_out, b_out, ln1_g, ln1_b, w1, b1, w2, b2,
           ln2_g, ln2_b, spatial_shapes, level_start_index):
    src = np.asarray(src, np.float32)
    pos = np.asarray(pos, np.float32)
    te = np.asarray(time_embed, np.float32)
    ref = np.asarray(reference_points, np.float32).reshape(2, LEN_IN, L)

    consts = _host_consts()

    woa_full = np.concatenate([np.asarray(w_off, np.float32)[:, _OAPERM],
                               np.asarray(w_attn, np.float32)[:, _OAPERM]],
                              axis=1)
    boa_full = np.concatenate([np.asarray(b_off, np.float32)[_OAPERM],
                               np.asarray(b_attn, np.float32)[_OAPERM]])[None, :]

    wval_p = np.asarray(w_val, np.float32)[:, _PERM]
    wout_r = np.asarray(w_out, np.float32)[_PERM, :]

    w2f = np.asarray(w2, np.float32)
    w2c = np.stack([w2f[k * 128:(k + 1) * 128] for k in range(8)],
                   axis=1).astype(np.float16)
    b1f = np.asarray(b1, np.float32)
    b1t = np.stack([b1f[k * 128:(k + 1) * 128] for k in range(8)], axis=1)
    rep16 = lambda v: np.repeat(
        np.asarray(v, np.float32)[None, :], 128, axis=0).astype(np.float16)

    shared = {
        'wvalP': _kchunk(wval_p),
        'bvalP': np.asarray(b_val, np.float32)[_PERM][None, :].astype(np.float16),
        'woa': _kchunk(woa_full),
        'boa': boa_full.astype(np.float16),
        'woutR': _kchunk(wout_r),
        'bout': np.asarray(b_out, np.float32)[None, :].astype(np.float16),
        'w1': _kchunk(np.asarray(w1, np.float32)),
        'b1t': b1t.astype(np.float32),
        'w2': w2c,
        'b2': np.asarray(b2, np.float32)[None, :].astype(np.float16),
        'ln1g': rep16(ln1_g),
        'ln2g': rep16(ln2_g),
        'ln2b': rep16(ln2_b),
        'cbias': np.repeat(np.array([[0., -1., -2., -3., -4., -5., EPS, 0.]],
                                    np.float32), 128, axis=0),
        **consts,
    }

    ln1b_f = np.asarray(ln1_b, np.float32)
    tscale = np.array(LEVEL_LENS, np.float32)

    in_maps = []
    for core in range(N_CORES):
        n, qr = core // 4, core % 4
        qs = slice(qr * QC, (qr + 1) * QC)
        m = dict(shared)
        # src^T k-chunked, full batch
        sT = src[n].T  # [256, 15360]
        m['srcTk'] = np.stack([sT[0:128], sT[128:256]], axis=1).astype(np.float16)
        m['srcTq'] = np.ascontiguousarray(m['srcTk'][:, :, qs])
        pT = pos[n, qs].T
        m['posTk'] = np.stack([pT[0:128], pT[128:256]], axis=1).astype(np.float16)
        m['teb'] = (te[n, qs] + ln1b_f[None, :]).astype(np.float16)

        ar = ref[n, qs] * tscale[None, :] - np.float32(0.5)  # [3840, 4] f32
        flr = np.floor(ar)
        m['arfq'] = (ar - flr + np.float32(2.0)).astype(np.float32)
        rows = (flr.astype(np.int64)
                + np.array(PSTARTS, np.int64)[None, :] - 2)  # [3840, 4]
        # gather idx order per macro: i = l*384 + j*128 + p
        wx = np.zeros((16, NMAC * (NIDX // 16)), np.int16)
        ridx = rows.reshape(NMAC, JT, 128, L)  # [m, j, p, l]
        for mm_ in range(NMAC):
            blk = np.transpose(ridx[mm_], (2, 0, 1)).reshape(NIDX)  # (l, j, p)
            wx[:, mm_ * (NIDX // 16):(mm_ + 1) * (NIDX // 16)] = (
                blk.reshape(NIDX // 16, 16).T.astype(np.int16))
        m['widx'] = np.tile(wx, (8, 1))
        in_maps.append(m)

    nc = _get_program()
    from concourse.bass_utils import run_bass_kernel_spmd
    res = run_bass_kernel_spmd(nc, in_maps, core_ids=list(range(N_CORES)))
    global LAST_RESULTS
    LAST_RESULTS = res
    if getattr(res, 'exec_time_ns', None):
        print('HW exec time:', res.exec_time_ns, 'ns')

    out = np.zeros((2, LEN_IN, D), np.float32)
    for core in range(N_CORES):
        n, qr = core // 4, core % 4
        out[n, qr * QC:(qr + 1) * QC] = res.results[core]['out_q'].astype(np.float32)
    return out


# revision 31
# speedup vs baseline: 1.5954x; 1.3022x over previous
"""Deformable transformer encoder layer on 8 Trainium2 NeuronCores (v2).

Sharding: core c handles batch c//4, query-quarter c%4 (3840 queries each).

v2 design (vs v1 baseline):
  - fp16 activations/weights throughout (DVE 2x modes, halved gather bytes).
  - value table columns permuted to (k2, h, u) so the per-level window
    multiply runs with a 3-4-dim AP whose innermost dims are stride-1 on
    both operands -> DVE 2x_1p mode.
  - one SWDGE dma_gather per 384-query macro-tile fetches all 4 levels'
    6-row windows (1536 indices, host-precomputed int16, SWDGE fixed
    overhead amortized 12x vs per-(tile,level) indirect DMAs).
  - hat weights relu(1-|x-r|) via ScalarE Abs/Relu with immediate biases;
    rsqrt(var) = Exp(-0.5*Ln(var+eps)) so every ScalarE function used
    ({Exp, Abs, Relu, Copy, Ln}) lives in one act table -> no reloads.
  - residuals (src +, x +) folded into the out-proj / FFN-w2 matmuls as
    identity-matrix matmuls accumulating in PSUM (no DVE residual adds,
    src never loaded untransposed).
  - all direct DMAs issued from gpsimd (25ns issue vs 565ns on sync).
"""
import os
import sys

sys.path.insert(0, '/opt/trn_rl_repo')

import dataclasses
import numpy as np
import ml_dtypes

import concourse.bass as bass
import concourse.mybir as mybir
from concourse.tile import TileContext
from concourse import library_config
from concourse.library_overlay import lower_extended_insts

# ---- tile drain workaround (this walrus rejects multi-wait drains) ----
import concourse.tile as _tile_mod
from concourse.tile_sem_assignment import tick_to_sem as _tick_to_sem


def _split_drain_and_barrier(self, tick_clock, wait_clock):
    gc = tick_clock.global_clock
    allocated = self.sems.allocated() if self.sems is not None else {}
    for proc, sem in sorted(allocated.items()):
        t = gc[proc]
        if t > 0:
            self.nc.sync.wait_ge(sem, _tick_to_sem(t, proc))
    self.nc.sync.drain()
    self.nc.all_engine_barrier()
    assert self.sems is not None
    popped = self.nc._tile_sem_poison_stack.pop()
    assert popped is self._sem_poison
    self.nc.clear_and_free_semaphores(list(self.sems.allocated().values()))
    self.nc.all_engine_barrier()


_tile_mod.TileContext._drain_and_barrier = _split_drain_and_barrier

_MAX_WAITS = 1
_wsplit_n = [0]


def _split_excess_waits(nc):
    """Walrus rejects instructions with >2 sem waits; move extras to nops."""
    for f in nc.m.functions:
        for bb in f.blocks:
            out = []
            for inst in list(bb.instructions):
                si = inst.sync_info
                waits = list(si.on_wait) if (si and si.on_wait) else []
                if len(waits) > _MAX_WAITS:
                    extra = waits[:-_MAX_WAITS]
                    keep = waits[-_MAX_WAITS:]
                    for j in range(0, len(extra), _MAX_WAITS):
                        _wsplit_n[0] += 1
                        nop = mybir.InstNoOp(name=f'wsplit-{_wsplit_n[0]}',
                                             ins=[], outs=[])
                        nop.engine = inst.engine
                        nop.sync_info = mybir.SyncInfo(
                            on_wait=extra[j:j + _MAX_WAITS], on_update=[])
                        out.append(nop)
                    inst.sync_info = mybir.SyncInfo(
                        on_wait=keep, on_update=list(si.on_update or []))
                out.append(inst)
            bb.instructions = out


f32 = mybir.dt.float32
fp16 = mybir.dt.float16
i16 = mybir.dt.int16
AF = mybir.ActivationFunctionType
OP = mybir.AluOpType

# ---- problem constants ----
D = 256
DF = 1024
H = 8
L = 4
NP = 4
LEVEL_LENS = (8192, 4096, 2048, 1024)
LEN_IN = 15360
N_CORES = 8
EPS = 1e-5

W = 6
PAD = 8
_starts = []
_acc = PAD
for _t in LEVEL_LENS:
    _starts.append(_acc)
    _acc += _t + PAD
PSTARTS = tuple(_starts)
VROWS = _acc               # 15400
QC = LEN_IN // 4           # 3840
NT_V = LEN_IN // 128       # 120
MACRO = 384                # queries per macro-tile
JT = MACRO // 128          # 3
NMAC = QC // MACRO         # 10
WIN = W * D                # 1536
NIDX = MACRO * L           # 1536 gather indices per macro
CUMS = [0]
for _t in LEVEL_LENS:
    CUMS.append(CUMS[-1] + _t)


def _bc(ap, dims, extra_off=0):
    """Replace the free dims of an AP with an explicit dim list."""
    ap2 = dataclasses.replace(
        ap, ap=[list(ap.ap[0])] + [list(d) for d in dims])
    if extra_off:
        ap2 = dataclasses.replace(ap2, offset=ap2.offset + extra_off)
    return ap2


def _dap(ap, dims, extra_off=0):
    """Replace the WHOLE AP dim list (incl. leading/partition dim)."""
    ap2 = dataclasses.replace(ap, ap=[list(d) for d in dims])
    if extra_off:
        ap2 = dataclasses.replace(ap2, offset=ap2.offset + extra_off)
    return ap2


def _build_program():
    G1_ONE = _FLAGS['g1_one']
    G2_ONE = _FLAGS['g2_one']
    B2_ZERO = _FLAGS['b2_zero']
    nc = bass.Bass(trn_type='TRN2')

    din = {}

    def I(name, shape, dt):
        din[name] = nc.dram_tensor(name, shape, dt, kind='ExternalInput')
        return din[name]

    srcTk = I('srcTk', [128, 2, LEN_IN], fp16)   # src^T, k-chunked (full batch)
    srcTq = I('srcTq', [128, 2, QC], fp16)       # src^T (this core's quarter)
    posTk = I('posTk', [128, 2, QC], fp16)       # pos^T (quarter)
    teb = I('teb', [QC, D], fp16)                # time_embed + ln1_b
    arfq = I('arfq', [QC, L], f32)               # frac(ar) + 2
    widx = I('widx', [128, NMAC * (NIDX // 16)], i16)

    ident = I('ident', [128, 128], fp16)
    ones1 = I('ones1', [1, 128], fp16)
    ires = I('ires', [128, 2, D], fp16)          # identity for PSUM residual
    wvalP = I('wvalP', [128, 2, D], fp16)
    bvalP = I('bvalP', [1, D], fp16)
    woa = I('woa', [128, 2, D], fp16)
    boa = I('boa', [1, D], fp16)
    woutR = I('woutR', [128, 2, D], fp16)
    bout = I('bout', [1, D], fp16)
    w1 = I('w1', [128, 2, DF], fp16)
    b1t = I('b1t', [128, 8], f32)
    w2 = I('w2', [128, 8, D], fp16)
    b2 = I('b2', [1, D], fp16)
    ln1g = I('ln1g', [128, D], fp16)
    ln2g = I('ln2g', [128, D], fp16)
    ln2b = I('ln2b', [128, D], fp16)
    cbias = I('cbias', [128, 8], f32)   # cols 0..5: -r ; col 6: eps

    out_q = nc.dram_tensor('out_q', [QC, D], fp16, kind='ExternalOutput')

    with TileContext(nc) as tc:
        with tc.tile_pool(name='cst', bufs=1) as cst, \
             tc.tile_pool(name='io', bufs=3) as io, \
             tc.tile_pool(name='wk', bufs=1) as wk, \
             tc.tile_pool(name='gat', bufs=2) as gat, \
             tc.tile_pool(name='p256', bufs=3, space='PSUM') as p256, \
             tc.tile_pool(name='ptr', bufs=2, space='PSUM') as ptr, \
             tc.tile_pool(name='php', bufs=2, space='PSUM') as php, \
             tc.tile_pool(name='dram', bufs=1, space='DRAM') as dram:

            nc.gpsimd.load_library(library_config.mlp)

            vtab = dram.tile([VROWS, D], fp16)

            def ctile(name, dt=fp16):
                t = cst.tile(list(din[name].shape), dt, tag=name)
                nc.sync.dma_start(
                    t[:], din[name][tuple(slice(None) for _ in din[name].shape)])
                return t

            identS = ctile('ident')
            ones1S = ctile('ones1')
            iresS = ctile('ires')
            wvalS = ctile('wvalP')
            bvalS = ctile('bvalP')
            woaS = ctile('woa')
            boaS = ctile('boa')
            woutS = ctile('woutR')
            boutS = ctile('bout')
            w1S = ctile('w1')
            b1tS = ctile('b1t', f32)
            w2S = ctile('w2')
            b2S = ctile('b2')
            ln1gS = ctile('ln1g')
            ln2gS = ctile('ln2g')
            ln2bS = ctile('ln2b')
            cbiasS = ctile('cbias', f32)

            # ---- zero pad rows of value table ----
            zpad = cst.tile([PAD, D], fp16, tag='zpad')
            nc.vector.memset(zpad[:], 0.0)
            nc.sync.dma_start(vtab[0:PAD, :], zpad[:])
            for lv in range(L):
                r0 = PSTARTS[lv] + LEVEL_LENS[lv]
                nc.sync.dma_start(vtab[r0:r0 + PAD, :], zpad[:])

            # ---- phase A: value projection ----
            for i in range(int(os.environ.get('K_NTV', NT_V))):
                r = i * 128
                lv = next(k for k in range(L) if r < CUMS[k + 1])
                prow = PSTARTS[lv] + (r - CUMS[lv])

                st = io.tile([128, 2, 128], fp16, tag='va_in')
                nc.sync.dma_start(st[:], srcTk[:, :, r:r + 128])
                vps = p256.tile([128, D], f32, tag='p256')
                nc.tensor.matmul(vps[:], st[:, 0, :], wvalS[:, 0, :],
                                 start=True, stop=False)
                nc.tensor.matmul(vps[:], st[:, 1, :], wvalS[:, 1, :],
                                 start=False, stop=False)
                nc.tensor.matmul(vps[:], ones1S[:1, :], bvalS[:1, :],
                                 start=False, stop=True)
                vb = wk.tile([128, D], fp16, tag='va_out', bufs=3)
                nc.scalar.activation(vb[:], vps[:], AF.Copy)
                nc.sync.dma_start(vtab[prow:prow + 128, :], vb[:])

            vwin = dataclasses.replace(
                vtab[:, :], ap=[[D, VROWS - W], [1, WIN]])

            # ---- phase B: macro-tiles ----
            for m in range(int(os.environ.get('K_NMAC', NMAC))):
                q0 = m * MACRO

                sTq = io.tile([128, 2, MACRO], fp16, tag='sTq')
                nc.sync.dma_start(sTq[:], srcTq[:, :, q0:q0 + MACRO])
                pTq = io.tile([128, 2, MACRO], fp16, tag='pTq')
                nc.sync.dma_start(pTq[:], posTk[:, :, q0:q0 + MACRO])
                tebS = io.tile([128, JT, D], fp16, tag='tebS')
                nc.sync.dma_start(
                    tebS[:],
                    _dap(teb[:, :], [[D, 128], [128 * D, JT], [1, D]],
                         extra_off=q0 * D))
                arfS = io.tile([128, JT, L], f32, tag='arfS')
                nc.sync.dma_start(
                    arfS[:],
                    _dap(arfq[:, :], [[L, 128], [128 * L, JT], [1, L]],
                         extra_off=q0 * L))
                wxS = io.tile([128, NIDX // 16], i16, tag='wxS')
                nc.sync.dma_start(
                    wxS[:], widx[:, m * (NIDX // 16):(m + 1) * (NIDX // 16)])

                # gather: all 4 levels x 3 subtiles, 6-row windows.
                # two calls of 768 idxs (SWDGE ring caps out between 1024
                # and 1536 descriptors per call)
                gw = gat.tile([128, L * JT, WIN], fp16, tag='gw')
                if os.environ.get('K_NOGATHER', '0') == '1':
                    nc.vector.memset(gw[:], 0.01)
                else:
                    half = NIDX // 2
                    for g2 in range(2):
                        nc.gpsimd.dma_gather(
                            out_ap=gw[:, g2 * (L * JT // 2):(g2 + 1) * (L * JT // 2), :],
                            in_ap=vwin,
                            idxs_ap=wxS[:, g2 * (half // 16):(g2 + 1) * (half // 16)],
                            num_idxs=half, num_idxs_reg=half,
                            elem_size=WIN, elem_step=D)

                # qT = (src + pos)^T
                qT = wk.tile([128, 2, MACRO], fp16, tag='qT', bufs=2)
                nc.vector.tensor_tensor(qT[:], sTq[:], pTq[:], OP.add)

                # off/attn projection per subtile
                oapss = []
                for j in range(JT):
                    oaps = p256.tile([128, D], f32, tag='p256')
                    nc.tensor.matmul(oaps[:], qT[:, 0, j * 128:(j + 1) * 128],
                                     woaS[:, 0, :], start=True, stop=False)
                    nc.tensor.matmul(oaps[:], qT[:, 1, j * 128:(j + 1) * 128],
                                     woaS[:, 1, :], start=False, stop=False)
                    nc.tensor.matmul(oaps[:], ones1S[:1, :], boaS[:1, :],
                                     start=False, stop=True)
                    oapss.append(oaps)

                # softmax over k=(l,p) per (j, h); attn cols are (l, p, h)
                exS = wk.tile([128, JT * 128], fp16, tag='exS')
                for j in range(JT):
                    nc.scalar.activation(exS[:, j * 128:(j + 1) * 128],
                                         oapss[j][:, 128:256], AF.Exp)
                zs = wk.tile([128, JT * H], f32, tag='zs')
                nc.vector.tensor_reduce(
                    zs[:],
                    _bc(exS[:, :], [[128, JT], [1, H], [8, 16]]),
                    mybir.AxisListType.X, OP.add)
                zr = wk.tile([128, JT * H], fp16, tag='zr')
                with nc.allow_low_precision(reason='softmax denom fp16 is ample'):
                    nc.vector.reciprocal(zr[:], zs[:])
                wn = wk.tile([128, JT * 128], fp16, tag='wn')
                nc.vector.tensor_tensor(
                    _bc(wn[:, :], [[128, JT], [8, 16], [1, H]]),
                    _bc(exS[:, :], [[128, JT], [8, 16], [1, H]]),
                    _bc(zr[:, :], [[H, JT], [0, 16], [1, H]]), OP.mult)

                # xq[p, (j,l,p4,h)] = off + arf  (off cols are (l, p4, h))
                xq = wk.tile([128, JT * 128], fp16, tag='xq')
                for j in range(JT):
                    nc.vector.tensor_tensor(
                        xq[:, j * 128:(j + 1) * 128],
                        oapss[j][:, 0:128],
                        _bc(arfS[:, :, :], [[1, L], [0, NP * H]],
                            extra_off=j * L), OP.add)

                # hat: h1[r] = relu(1 - |xq - r|)
                xqr = wk.tile([128, W * JT * 128], fp16, tag='xqr')
                for r in range(W):
                    nc.vector.tensor_scalar(
                        xqr[:, r * JT * 128:(r + 1) * JT * 128], xq[:],
                        float(r), None, OP.subtract)
                da = wk.tile([128, W * JT * 128], fp16, tag='da')
                nc.scalar.activation(da[:], xqr[:], AF.Abs)
                h1 = wk.tile([128, W * JT * 128], fp16, tag='h1')
                nc.scalar.activation(h1[:], da[:], AF.Relu, bias=1.0,
                                     scale=-1.0)

                # c3p[(j,l,r,p4,h)] = h1[(r,j,l,p4,h)] * wn[(j,l,p4,h)]
                c3p = wk.tile([128, W * JT * 128], fp16, tag='c3p')
                nc.vector.tensor_tensor(
                    _bc(c3p[:, :], [[192, JT * L], [32, W], [1, NP * H]]),
                    _bc(h1[:, :], [[32, JT * L], [384, W], [1, NP * H]]),
                    _bc(wn[:, :], [[32, JT * L], [0, W], [1, NP * H]]),
                    OP.mult)

                # fold sampling points: A[(j,l,r), u, h] = p_u + p_{u+2}
                Af = wk.tile([128, W * JT * 64], fp16, tag='Af')
                nc.vector.tensor_tensor(
                    Af[:],
                    _bc(c3p[:, :], [[32, JT * L * W], [8, 2], [1, H]]),
                    _bc(c3p[:, :], [[32, JT * L * W], [8, 2], [1, H]],
                        extra_off=16), OP.add)
                # c3d[(j,l,r), h, u'] = A[..., 0, h] + A[..., 1, h], dup x2
                c3d = wk.tile([128, JT * 384], fp16, tag='c3d')
                for up in range(2):
                    nc.vector.tensor_tensor(
                        _bc(c3d[:, :], [[16, JT * L * W], [2, H]],
                            extra_off=up),
                        _bc(Af[:, :], [[16, JT * L * W], [1, H]]),
                        _bc(Af[:, :], [[16, JT * L * W], [1, H]],
                            extra_off=8), OP.add)

                # sampling reduce per subtile
                attjs = []
                for j in range(JT):
                    tt = gat.tile([128, L * W * D], fp16, tag='tt', bufs=1)
                    nc.vector.tensor_tensor(
                        tt[:],
                        _bc(gw[:, :, :], [[JT * WIN, L], [D, W], [1, D]],
                            extra_off=j * WIN),
                        _bc(c3d[:, :], [[16, L * W], [0, 16], [1, 16]],
                            extra_off=j * 384), OP.mult)
                    Y = wk.tile([128, L * 3 * D], fp16, tag='Yf')
                    nc.vector.tensor_tensor(
                        Y[:],
                        _bc(tt[:, :], [[6 * D, L], [D, 3], [1, D]]),
                        _bc(tt[:, :], [[6 * D, L], [D, 3], [1, D]],
                            extra_off=3 * D), OP.add)
                    Zt = wk.tile([128, L * D], fp16, tag='Zf')
                    nc.vector.tensor_tensor(
                        Zt[:],
                        _bc(Y[:, :], [[3 * D, L], [1, D]]),
                        _bc(Y[:, :], [[3 * D, L], [1, D]], extra_off=D),
                        OP.add)
                    at0 = wk.tile([128, L * D], fp16, tag='at0')
                    nc.vector.tensor_tensor(
                        at0[:], Zt[:],
                        _bc(Y[:, :], [[3 * D, L], [1, D]], extra_off=2 * D),
                        OP.add)
                    s01 = wk.tile([128, 2 * D], fp16, tag='s01')
                    nc.vector.tensor_tensor(s01[:], at0[:, 0:2 * D],
                                            at0[:, 2 * D:4 * D], OP.add)
                    attj = wk.tile([128, D], fp16, tag='attj', bufs=3)
                    nc.vector.tensor_tensor(attj[:], s01[:, 0:D],
                                            s01[:, D:2 * D], OP.add)
                    attjs.append(attj)

                # out-projection + src residual (PSUM) + LN1
                xx = wk.tile([128, JT * D], fp16, tag='xx', bufs=2)
                xT = wk.tile([128, 2, MACRO], fp16, tag='xT', bufs=2)
                s2f = wk.tile([128, JT * D], fp16, tag='s2f', bufs=2)
                msJ = wk.tile([128, JT], f32, tag='msJ', bufs=2)
                ssqJ = wk.tile([128, JT], f32, tag='ssqJ', bufs=2)
                for j in range(JT):
                    attT = wk.tile([128, 2, 128], fp16, tag='attT', bufs=2)
                    tp2 = ptr.tile([128, 256], fp16, tag='tr')
                    for c in range(2):
                        nc.tensor.transpose(
                            tp2[:, c * 128:(c + 1) * 128],
                            attjs[j][:, c * 128:(c + 1) * 128],
                            identS[:])
                    nc.scalar.activation(
                        attT[:].rearrange("p a b -> p (a b)"), tp2[:], AF.Copy)
                    s2ps = p256.tile([128, D], f32, tag='p256')
                    nc.tensor.matmul(s2ps[:], attT[:, 0, :], woutS[:, 0, :],
                                     start=True, stop=False)
                    nc.tensor.matmul(s2ps[:], attT[:, 1, :], woutS[:, 1, :],
                                     start=False, stop=False)
                    for c in range(2):
                        nc.tensor.matmul(
                            s2ps[:],
                            sTq[:, c, j * 128:(j + 1) * 128],
                            iresS[:, c, :], start=False, stop=False)
                    nc.tensor.matmul(s2ps[:], ones1S[:1, :], boutS[:1, :],
                                     start=False, stop=True)
                    nc.scalar.activation(s2f[:, j * D:(j + 1) * D], s2ps[:],
                                         AF.Copy, accum_out=msJ[:, j:j + 1])
                    sq = wk.tile([128, D], fp16, tag='sq')
                    nc.scalar.activation(sq[:], s2ps[:], AF.Square,
                                         accum_out=ssqJ[:, j:j + 1])
                # batched LN1 stats (FD=JT)
                muJ = wk.tile([128, JT], f32, tag='muJ', bufs=2)
                nc.vector.tensor_scalar(muJ[:], msJ[:], 1.0 / D, None, OP.mult)
                mu2J = wk.tile([128, JT], f32, tag='mu2J')
                nc.vector.tensor_tensor(mu2J[:], muJ[:], muJ[:], OP.mult)
                varJ = wk.tile([128, JT], f32, tag='varJ')
                nc.vector.scalar_tensor_tensor(
                    varJ[:], ssqJ[:], 1.0 / D, mu2J[:], OP.mult, OP.subtract)
                lnvJ = wk.tile([128, JT], f32, tag='lnvJ')
                nc.scalar.activation(lnvJ[:], varJ[:], AF.Ln,
                                     bias=cbiasS[:, 6:7], scale=1.0)
                rsdJ = wk.tile([128, JT], f32, tag='rsdJ', bufs=2)
                nc.scalar.activation(rsdJ[:], lnvJ[:], AF.Exp, bias=0.0,
                                     scale=-0.5)
                for j in range(JT):
                    xn = wk.tile([128, D], fp16, tag='xn')
                    nc.vector.tensor_scalar(xn[:], s2f[:, j * D:(j + 1) * D],
                                            muJ[:, j:j + 1], rsdJ[:, j:j + 1],
                                            OP.subtract, OP.mult)
                    if G1_ONE:
                        nc.vector.tensor_tensor(xx[:, j * D:(j + 1) * D],
                                                xn[:], tebS[:, j, :], OP.add)
                    else:
                        t1 = wk.tile([128, D], fp16, tag='t1')
                        nc.vector.tensor_tensor(t1[:], xn[:], ln1gS[:, :],
                                                OP.mult)
                        nc.vector.tensor_tensor(xx[:, j * D:(j + 1) * D],
                                                t1[:], tebS[:, j, :], OP.add)
                    # transpose xx_j for FFN + residual
                    tp3 = ptr.tile([128, 256], fp16, tag='tr')
                    for c in range(2):
                        nc.tensor.transpose(
                            tp3[:, c * 128:(c + 1) * 128],
                            xx[:, j * D + c * 128:j * D + (c + 1) * 128],
                            identS[:])
                    nc.scalar.activation(
                        _bc(xT[:, :, :], [[MACRO, 2], [1, 128]],
                            extra_off=j * 128), tp3[:], AF.Copy)

                # FFN w1 (all subtiles at once)
                hbT = wk.tile([128, 8, MACRO], fp16, tag='hbT', bufs=2)
                for k in range(8):
                    hps = php.tile([128, MACRO], f32, tag='hps')
                    nc.tensor.matmul(hps[:], w1S[:, 0, k * 128:(k + 1) * 128],
                                     xT[:, 0, :], start=True, stop=False)
                    nc.tensor.matmul(hps[:], w1S[:, 1, k * 128:(k + 1) * 128],
                                     xT[:, 1, :], start=False, stop=True)
                    nc.scalar.activation(hbT[:, k, :], hps[:], AF.Relu,
                                         bias=b1tS[:, k:k + 1])

                # FFN w2 + xx residual + LN2
                of = wk.tile([128, JT * D], fp16, tag='of', bufs=2)
                o2f = wk.tile([128, JT * D], fp16, tag='o2f', bufs=2)
                ms2J = wk.tile([128, JT], f32, tag='ms2J', bufs=2)
                ssq2J = wk.tile([128, JT], f32, tag='ssq2J', bufs=2)
                for j in range(JT):
                    o2ps = p256.tile([128, D], f32, tag='p256')
                    for k in range(8):
                        nc.tensor.matmul(
                            o2ps[:], hbT[:, k, j * 128:(j + 1) * 128],
                            w2S[:, k, :], start=(k == 0), stop=False)
                    for c in range(2):
                        nc.tensor.matmul(
                            o2ps[:], xT[:, c, j * 128:(j + 1) * 128],
                            iresS[:, c, :], start=False, stop=False)
                    nc.tensor.matmul(o2ps[:], ones1S[:1, :], b2S[:1, :],
                                     start=False, stop=True)
                    nc.scalar.activation(o2f[:, j * D:(j + 1) * D], o2ps[:],
                                         AF.Copy, accum_out=ms2J[:, j:j + 1])
                    sq2 = wk.tile([128, D], fp16, tag='sq2')
                    nc.scalar.activation(sq2[:], o2ps[:], AF.Square,
                                         accum_out=ssq2J[:, j:j + 1])
                mu_2J = wk.tile([128, JT], f32, tag='mu_2J', bufs=2)
                nc.vector.tensor_scalar(mu_2J[:], ms2J[:], 1.0 / D, None,
                                        OP.mult)
                mu22J = wk.tile([128, JT], f32, tag='mu22J')
                nc.vector.tensor_tensor(mu22J[:], mu_2J[:], mu_2J[:], OP.mult)
                var2J = wk.tile([128, JT], f32, tag='var2J')
                nc.vector.scalar_tensor_tensor(
                    var2J[:], ssq2J[:], 1.0 / D, mu22J[:], OP.mult,
                    OP.subtract)
                lnv2J = wk.tile([128, JT], f32, tag='lnv2J')
                nc.scalar.activation(lnv2J[:], var2J[:], AF.Ln,
                                     bias=cbiasS[:, 6:7], scale=1.0)
                rsd2J = wk.tile([128, JT], f32, tag='rsd2J', bufs=2)
                nc.scalar.activation(rsd2J[:], lnv2J[:], AF.Exp, bias=0.0,
                                     scale=-0.5)
                for j in range(JT):
                    if G2_ONE and B2_ZERO:
                        nc.vector.tensor_scalar(
                            of[:, j * D:(j + 1) * D],
                            o2f[:, j * D:(j + 1) * D],
                            mu_2J[:, j:j + 1], rsd2J[:, j:j + 1],
                            OP.subtract, OP.mult)
                    else:
                        xn2 = wk.tile([128, D], fp16, tag='xn2')
                        nc.vector.tensor_scalar(
                            xn2[:], o2f[:, j * D:(j + 1) * D],
                            mu_2J[:, j:j + 1], rsd2J[:, j:j + 1],
                            OP.subtract, OP.mult)
                        t2 = wk.tile([128, D], fp16, tag='t2')
                        nc.vector.tensor_tensor(t2[:], xn2[:], ln2gS[:, :],
                                                OP.mult)
                        nc.vector.tensor_tensor(of[:, j * D:(j + 1) * D],
                                                t2[:], ln2bS[:, :], OP.add)

                nc.sync.dma_start(
                    _dap(out_q[:, :], [[D, 128], [128 * D, JT], [1, D]],
                         extra_off=q0 * D),
                    of[:])

    if os.environ.get('K_NOSPLIT', '0') != '1':
        _split_excess_waits(nc)
    lower_extended_insts(nc)
    return nc


_PROG = None
LAST_RESULTS = None
_FLAGS = {'g1_one': False, 'g2_one': False, 'b2_zero': False}


def _get_program():
    global _PROG
    if _PROG is None:
        _PROG = _build_program()
    return _PROG


# column permutation: new col (k2, h, u) <- old col h*32 + k2*2 + u
_PERM = np.zeros(D, np.int64)
for _k2 in range(16):
    for _h in range(H):
        for _u in range(2):
            _PERM[_k2 * 16 + _h * 2 + _u] = _h * 32 + _k2 * 2 + _u

# off/attn projection column permutation: new col (l, p, h) <- old (h, l, p)
_OAPERM = np.zeros(128, np.int64)
for _l in range(L):
    for _p in range(NP):
        for _h in range(H):
            _OAPERM[_l * 32 + _p * 8 + _h] = _h * 16 + _l * 4 + _p


def _host_consts():
    c = {}
    c['ident'] = np.eye(128, dtype=np.float16)
    c['ones1'] = np.ones((1, 128), np.float16)
    ir = np.zeros((128, 2, D), np.float16)
    for p in range(128):
        ir[p, 0, p] = 1.0
        ir[p, 1, 128 + p] = 1.0
    c['ires'] = ir
    return c


def _kchunk(w):
    """[256, X] f32 -> [128, 2, X] fp16"""
    w = np.asarray(w, np.float32)
    return np.stack([w[0:128], w[128:256]], axis=1).astype(np.float16)


def kernel(src, pos, time_embed, reference_points, w_off, b_off, w_attn,
           b_attn, w_val, b_val, w_out, b_out, ln1_g, ln1_b, w1, b1, w2, b2,
           ln2_g, ln2_b, spatial_shapes, level_start_index):
    src = np.asarray(src, np.float32)
    pos = np.asarray(pos, np.float32)
    te = np.asarray(time_embed, np.float32)
    ref = np.asarray(reference_points, np.float32).reshape(2, LEN_IN, L)

    consts = _host_consts()
    _FLAGS['g1_one'] = bool(np.allclose(np.asarray(ln1_g, np.float32), 1.0))
    _FLAGS['g2_one'] = bool(np.allclose(np.asarray(ln2_g, np.float32), 1.0))
    _FLAGS['b2_zero'] = bool(np.allclose(np.asarray(ln2_b, np.float32), 0.0))

    woa_full = np.concatenate([np.asarray(w_off, np.float32)[:, _OAPERM],
                               np.asarray(w_attn, np.float32)[:, _OAPERM]],
                              axis=1)
    boa_full = np.concatenate([np.asarray(b_off, np.float32)[_OAPERM],
                               np.asarray(b_attn, np.float32)[_OAPERM]])[None, :]

    wval_p = np.asarray(w_val, np.float32)[:, _PERM]
    wout_r = np.asarray(w_out, np.float32)[_PERM, :]

    w2f = np.asarray(w2, np.float32)
    w2c = np.stack([w2f[k * 128:(k + 1) * 128] for k in range(8)],
                   axis=1).astype(np.float16)
    b1f = np.asarray(b1, np.float32)
    b1t = np.stack([b1f[k * 128:(k + 1) * 128] for k in range(8)], axis=1)
    rep16 = lambda v: np.repeat(
        np.asarray(v, np.float32)[None, :], 128, axis=0).astype(np.float16)

    shared = {
        'wvalP': _kchunk(wval_p),
        'bvalP': np.asarray(b_val, np.float32)[_PERM][None, :].astype(np.float16),
        'woa': _kchunk(woa_full),
        'boa': boa_full.astype(np.float16),
        'woutR': _kchunk(wout_r),
        'bout': np.asarray(b_out, np.float32)[None, :].astype(np.float16),
        'w1': _kchunk(np.asarray(w1, np.float32)),
        'b1t': b1t.astype(np.float32),
        'w2': w2c,
        'b2': np.asarray(b2, np.float32)[None, :].astype(np.float16),
        'ln1g': rep16(ln1_g),
        'ln2g': rep16(ln2_g),
        'ln2b': rep16(ln2_b),
        'cbias': np.repeat(np.array([[0., -1., -2., -3., -4., -5., EPS, 0.]],
                                    np.float32), 128, axis=0),
        **consts,
    }

    ln1b_f = np.asarray(ln1_b, np.float32)
    tscale = np.array(LEVEL_LENS, np.float32)

    in_maps = []
    for core in range(N_CORES):
        n, qr = core // 4, core % 4
        qs = slice(qr * QC, (qr + 1) * QC)
        m = dict(shared)
        # src^T k-chunked, full batch
        sT = src[n].T  # [256, 15360]
        m['srcTk'] = np.stack([sT[0:128], sT[128:256]], axis=1).astype(np.float16)
        m['srcTq'] = np.ascontiguousarray(m['srcTk'][:, :, qs])
        pT = pos[n, qs].T
        m['posTk'] = np.stack([pT[0:128], pT[128:256]], axis=1).astype(np.float16)
        m['teb'] = (te[n, qs] + ln1b_f[None, :]).astype(np.float16)

        ar = ref[n, qs] * tscale[None, :] - np.float32(0.5)  # [3840, 4] f32
        flr = np.floor(ar)
        m['arfq'] = (ar - flr + np.float32(2.0)).astype(np.float32)
        rows = (flr.astype(np.int64)
                + np.array(PSTARTS, np.int64)[None, :] - 2)  # [3840, 4]
        # gather idx order per macro: i = l*384 + j*128 + p
        wx = np.zeros((16, NMAC * (NIDX // 16)), np.int16)
        ridx = rows.reshape(NMAC, JT, 128, L)  # [m, j, p, l]
        for mm_ in range(NMAC):
            blk = np.transpose(ridx[mm_], (2, 0, 1)).reshape(NIDX)  # (l, j, p)
            wx[:, mm_ * (NIDX // 16):(mm_ + 1) * (NIDX // 16)] = (
                blk.reshape(NIDX // 16, 16).T.astype(np.int16))
        m['widx'] = np.tile(wx, (8, 1))
        in_maps.append(m)

    nc = _get_program()
    from concourse.bass_utils import run_bass_kernel_spmd
    res = run_bass_kernel_spmd(nc, in_maps, core_ids=list(range(N_CORES)))
    global LAST_RESULTS
    LAST_RESULTS = res
    if getattr(res, 'exec_time_ns', None):
        print('HW exec time:', res.exec_time_ns, 'ns')

    out = np.zeros((2, LEN_IN, D), np.float32)
    for core in range(N_CORES):
        n, qr = core // 4, core % 4
        out[n, qr * QC:(qr + 1) * QC] = res.results[core]['out_q'].astype(np.float32)
    return out


# revision 35
# speedup vs baseline: 2.3683x; 1.4845x over previous
"""Deformable transformer encoder layer on 8 Trainium2 NeuronCores (v2).

Sharding: core c handles batch c//4, query-quarter c%4 (3840 queries each).

v2 design (vs v1 baseline):
  - fp16 activations/weights throughout (DVE 2x modes, halved gather bytes).
  - value table columns permuted to (k2, h, u) so the per-level window
    multiply runs with a 3-4-dim AP whose innermost dims are stride-1 on
    both operands -> DVE 2x_1p mode.
  - one SWDGE dma_gather per 384-query macro-tile fetches all 4 levels'
    6-row windows (1536 indices, host-precomputed int16, SWDGE fixed
    overhead amortized 12x vs per-(tile,level) indirect DMAs).
  - hat weights relu(1-|x-r|) via ScalarE Abs/Relu with immediate biases;
    rsqrt(var) = Exp(-0.5*Ln(var+eps)) so every ScalarE function used
    ({Exp, Abs, Relu, Copy, Ln}) lives in one act table -> no reloads.
  - residuals (src +, x +) folded into the out-proj / FFN-w2 matmuls as
    identity-matrix matmuls accumulating in PSUM (no DVE residual adds,
    src never loaded untransposed).
  - all direct DMAs issued from gpsimd (25ns issue vs 565ns on sync).
"""
import os
import sys

sys.path.insert(0, '/opt/trn_rl_repo')

import dataclasses
import numpy as np
import ml_dtypes

import concourse.bass as bass
import concourse.mybir as mybir
from concourse.tile import TileContext
from concourse import library_config
from concourse.library_overlay import lower_extended_insts

# ---- tile drain workaround (this walrus rejects multi-wait drains) ----
import concourse.tile as _tile_mod
from concourse.tile_sem_assignment import tick_to_sem as _tick_to_sem


def _split_drain_and_barrier(self, tick_clock, wait_clock):
    gc = tick_clock.global_clock
    allocated = self.sems.allocated() if self.sems is not None else {}
    for proc, sem in sorted(allocated.items()):
        t = gc[proc]
        if t > 0:
            self.nc.sync.wait_ge(sem, _tick_to_sem(t, proc))
    self.nc.sync.drain()
    self.nc.all_engine_barrier()
    assert self.sems is not None
    popped = self.nc._tile_sem_poison_stack.pop()
    assert popped is self._sem_poison
    self.nc.clear_and_free_semaphores(list(self.sems.allocated().values()))
    self.nc.all_engine_barrier()


_tile_mod.TileContext._drain_and_barrier = _split_drain_and_barrier

_MAX_WAITS = 1
_wsplit_n = [0]


def _split_excess_waits(nc):
    """Walrus rejects instructions with >2 sem waits; move extras to nops."""
    for f in nc.m.functions:
        for bb in f.blocks:
            out = []
            for inst in list(bb.instructions):
                si = inst.sync_info
                waits = list(si.on_wait) if (si and si.on_wait) else []
                if len(waits) > _MAX_WAITS:
                    extra = waits[:-_MAX_WAITS]
                    keep = waits[-_MAX_WAITS:]
                    for j in range(0, len(extra), _MAX_WAITS):
                        _wsplit_n[0] += 1
                        nop = mybir.InstNoOp(name=f'wsplit-{_wsplit_n[0]}',
                                             ins=[], outs=[])
                        nop.engine = inst.engine
                        nop.sync_info = mybir.SyncInfo(
                            on_wait=extra[j:j + _MAX_WAITS], on_update=[])
                        out.append(nop)
                    inst.sync_info = mybir.SyncInfo(
                        on_wait=keep, on_update=list(si.on_update or []))
                out.append(inst)
            bb.instructions = out


f32 = mybir.dt.float32
fp16 = mybir.dt.float16
i16 = mybir.dt.int16
AF = mybir.ActivationFunctionType
OP = mybir.AluOpType

# ---- problem constants ----
D = 256
DF = 1024
H = 8
L = 4
NP = 4
LEVEL_LENS = (8192, 4096, 2048, 1024)
LEN_IN = 15360
N_CORES = 8
EPS = 1e-5

W = 6
PAD = 8
_starts = []
_acc = PAD
for _t in LEVEL_LENS:
    _starts.append(_acc)
    _acc += _t + PAD
PSTARTS = tuple(_starts)
VROWS = _acc               # 15400
QC = LEN_IN // 4           # 3840
NT_V = LEN_IN // 128       # 120
MACRO = 384                # queries per macro-tile
JT = MACRO // 128          # 3
NMAC = QC // MACRO         # 10
WIN = W * D                # 1536
NIDX = MACRO * L           # 1536 gather indices per macro
CUMS = [0]
for _t in LEVEL_LENS:
    CUMS.append(CUMS[-1] + _t)


def _bc(ap, dims, extra_off=0):
    """Replace the free dims of an AP with an explicit dim list."""
    ap2 = dataclasses.replace(
        ap, ap=[list(ap.ap[0])] + [list(d) for d in dims])
    if extra_off:
        ap2 = dataclasses.replace(ap2, offset=ap2.offset + extra_off)
    return ap2


def _dap(ap, dims, extra_off=0):
    """Replace the WHOLE AP dim list (incl. leading/partition dim)."""
    ap2 = dataclasses.replace(ap, ap=[list(d) for d in dims])
    if extra_off:
        ap2 = dataclasses.replace(ap2, offset=ap2.offset + extra_off)
    return ap2


def _build_program():
    G1_ONE = _FLAGS['g1_one']
    G2_ONE = _FLAGS['g2_one']
    B2_ZERO = _FLAGS['b2_zero']
    nc = bass.Bass(trn_type='TRN2')

    din = {}

    def I(name, shape, dt):
        din[name] = nc.dram_tensor(name, shape, dt, kind='ExternalInput')
        return din[name]

    srcTk = I('srcTk', [128, 2, LEN_IN], fp16)   # src^T, k-chunked (full batch)
    srcTq = I('srcTq', [128, 2, QC], fp16)       # src^T (this core's quarter)
    posTk = I('posTk', [128, 2, QC], fp16)       # pos^T (quarter)
    teb = I('teb', [QC, D], fp16)                # time_embed + ln1_b
    arfq = I('arfq', [QC, L], f32)               # frac(ar) + 2
    widx = I('widx', [128, NMAC * (NIDX // 16)], i16)

    ident = I('ident', [128, 128], fp16)
    ones1 = I('ones1', [1, 128], fp16)
    ires = I('ires', [128, 2, D], fp16)          # identity for PSUM residual
    wvalP = I('wvalP', [128, 2, D], fp16)
    bvalP = I('bvalP', [1, D], fp16)
    woa = I('woa', [128, 2, D], fp16)
    boa = I('boa', [1, D], fp16)
    woutR = I('woutR', [128, 2, D], fp16)
    bout = I('bout', [1, D], fp16)
    w1 = I('w1', [128, 2, DF], fp16)
    b1t = I('b1t', [128, 8], f32)
    w2 = I('w2', [128, 8, D], fp16)
    b2 = I('b2', [1, D], fp16)
    ln1g = I('ln1g', [128, D], fp16)
    ln2g = I('ln2g', [128, D], fp16)
    ln2b = I('ln2b', [128, D], fp16)
    cbias = I('cbias', [128, 8], f32)   # cols 0..5: -r ; col 6: eps

    out_q = nc.dram_tensor('out_q', [QC, D], fp16, kind='ExternalOutput')

    with TileContext(nc) as tc:
        with tc.tile_pool(name='cst', bufs=1) as cst, \
             tc.tile_pool(name='pin', bufs=NMAC) as pin, \
             tc.tile_pool(name='io', bufs=3) as io, \
             tc.tile_pool(name='wk', bufs=1) as wk, \
             tc.tile_pool(name='gat', bufs=2) as gat, \
             tc.tile_pool(name='p256', bufs=2, space='PSUM') as p256, \
             tc.tile_pool(name='pva', bufs=2, space='PSUM') as pva, \
             tc.tile_pool(name='ptr', bufs=2, space='PSUM') as ptr, \
             tc.tile_pool(name='php', bufs=2, space='PSUM') as php, \
             tc.tile_pool(name='dram', bufs=1, space='DRAM') as dram:

            nc.gpsimd.load_library(library_config.mlp)

            vtab = dram.tile([VROWS, D], fp16)

            def ctile(name, dt=fp16):
                t = cst.tile(list(din[name].shape), dt, tag=name)
                nc.sync.dma_start(
                    t[:], din[name][tuple(slice(None) for _ in din[name].shape)])
                return t

            identS = ctile('ident')
            ones1S = ctile('ones1')
            iresS = ctile('ires')
            wvalS = ctile('wvalP')
            bvalS = ctile('bvalP')
            woaS = ctile('woa')
            boaS = ctile('boa')
            woutS = ctile('woutR')
            boutS = ctile('bout')
            w1S = ctile('w1')
            b1tS = ctile('b1t', f32)
            w2S = ctile('w2')
            b2S = ctile('b2')
            ln1gS = ctile('ln1g')
            ln2gS = ctile('ln2g')
            ln2bS = ctile('ln2b')
            cbiasS = ctile('cbias', f32)

            # ---- zero pad rows of value table ----
            zpad = cst.tile([PAD, D], fp16, tag='zpad')
            nc.vector.memset(zpad[:], 0.0)
            nc.sync.dma_start(vtab[0:PAD, :], zpad[:])
            for lv in range(L):
                r0 = PSTARTS[lv] + LEVEL_LENS[lv]
                nc.sync.dma_start(vtab[r0:r0 + PAD, :], zpad[:])

            NM = int(os.environ.get('K_NMAC', NMAC))

            # ================= FRONT phase (no vtab dependency) =========
            sTqs, c3ds = [], []
            for m in range(NM):
                q0 = m * MACRO
                sTq = pin.tile([128, 2, MACRO], fp16, tag='sTq')
                nc.gpsimd.dma_start(sTq[:], srcTq[:, :, q0:q0 + MACRO])
                pTq = io.tile([128, 2, MACRO], fp16, tag='pTq')
                nc.gpsimd.dma_start(pTq[:], posTk[:, :, q0:q0 + MACRO])
                arfS = io.tile([128, JT, L], f32, tag='arfS')
                nc.gpsimd.dma_start(
                    arfS[:],
                    _dap(arfq[:, :], [[L, 128], [128 * L, JT], [1, L]],
                         extra_off=q0 * L))
                sTqs.append(sTq)

                # qT = (src + pos)^T
                qT = wk.tile([128, 2, MACRO], fp16, tag='qT', bufs=2)
                nc.vector.tensor_tensor(qT[:], sTq[:], pTq[:], OP.add)

                exS = wk.tile([128, JT * 128], fp16, tag='exS', bufs=2)
                xq = wk.tile([128, JT * 128], fp16, tag='xq', bufs=2)
                for j in range(JT):
                    oaps = p256.tile([128, D], f32, tag='p256')
                    nc.tensor.matmul(oaps[:], qT[:, 0, j * 128:(j + 1) * 128],
                                     woaS[:, 0, :], start=True, stop=False)
                    nc.tensor.matmul(oaps[:], qT[:, 1, j * 128:(j + 1) * 128],
                                     woaS[:, 1, :], start=False, stop=False)
                    nc.tensor.matmul(oaps[:], ones1S[:1, :], boaS[:1, :],
                                     start=False, stop=True)
                    nc.scalar.activation(exS[:, j * 128:(j + 1) * 128],
                                         oaps[:, 128:256], AF.Exp)
                    nc.vector.tensor_tensor(
                        xq[:, j * 128:(j + 1) * 128],
                        oaps[:, 0:128],
                        _bc(arfS[:, :, :], [[1, L], [0, NP * H]],
                            extra_off=j * L), OP.add)

                # softmax over k=(l,p) per (j, h); attn cols are (l, p, h)
                zs = wk.tile([128, JT * H], f32, tag='zs', bufs=2)
                nc.vector.tensor_reduce(
                    zs[:],
                    _bc(exS[:, :], [[128, JT], [1, H], [8, 16]]),
                    mybir.AxisListType.X, OP.add)
                zr = wk.tile([128, JT * H], fp16, tag='zr', bufs=2)
                with nc.allow_low_precision(reason='softmax denom fp16 ample'):
                    nc.vector.reciprocal(zr[:], zs[:])
                wn = wk.tile([128, JT * 128], fp16, tag='wn', bufs=2)
                nc.vector.tensor_tensor(
                    _bc(wn[:, :], [[128, JT], [8, 16], [1, H]]),
                    _bc(exS[:, :], [[128, JT], [8, 16], [1, H]]),
                    _bc(zr[:, :], [[H, JT], [0, 16], [1, H]]), OP.mult)

                # hat: h1[r] = relu(1 - |xq - r|)
                xqr = wk.tile([128, W * JT * 128], fp16, tag='xqr')
                for r in range(W):
                    nc.vector.tensor_scalar(
                        xqr[:, r * JT * 128:(r + 1) * JT * 128], xq[:],
                        float(r), None, OP.subtract)
                nc.scalar.activation(xqr[:], xqr[:], AF.Abs)
                nc.scalar.activation(xqr[:], xqr[:], AF.Relu, bias=1.0,
                                     scale=-1.0)
                h1 = xqr

                # c3p[(j,l,r,p4,h)] = h1[(r,j,l,p4,h)] * wn[(j,l,p4,h)]
                c3p = wk.tile([128, W * JT * 128], fp16, tag='c3p')
                nc.vector.tensor_tensor(
                    _bc(c3p[:, :], [[192, JT * L], [32, W], [1, NP * H]]),
                    _bc(h1[:, :], [[32, JT * L], [384, W], [1, NP * H]]),
                    _bc(wn[:, :], [[32, JT * L], [0, W], [1, NP * H]]),
                    OP.mult)

                # fold sampling points: A[(j,l,r), u, h] = p_u + p_{u+2}
                Af = wk.tile([128, W * JT * 64], fp16, tag='Af')
                nc.vector.tensor_tensor(
                    Af[:],
                    _bc(c3p[:, :], [[32, JT * L * W], [8, 2], [1, H]]),
                    _bc(c3p[:, :], [[32, JT * L * W], [8, 2], [1, H]],
                        extra_off=16), OP.add)
                # c3d[(j,l,r), h, u'] = A[..., 0, h] + A[..., 1, h], dup x2
                c3d = pin.tile([128, JT * 384], fp16, tag='c3d')
                for up in range(2):
                    nc.vector.tensor_tensor(
                        _bc(c3d[:, :], [[16, JT * L * W], [2, H]],
                            extra_off=up),
                        _bc(Af[:, :], [[16, JT * L * W], [1, H]]),
                        _bc(Af[:, :], [[16, JT * L * W], [1, H]],
                            extra_off=8), OP.add)
                c3ds.append(c3d)

            # ================= PHASE A: value projection ================
            for i in range(0, int(os.environ.get('K_NTV', NT_V)), 2):
                r = i * 128
                lv = next(k for k in range(L) if r < CUMS[k + 1])
                prow = PSTARTS[lv] + (r - CUMS[lv])
                # two row-tiles per DMA; stay within one level (tiles 2-align)
                st = io.tile([128, 2, 256], fp16, tag='va_in')
                nc.sync.dma_start(st[:], srcTk[:, :, r:r + 256])
                vb = wk.tile([128, 2, D], fp16, tag='va_out', bufs=3)
                for t in range(2):
                    vps = pva.tile([128, D], f32, tag='pva')
                    nc.tensor.matmul(vps[:], st[:, 0, t * 128:(t + 1) * 128],
                                     wvalS[:, 0, :], start=True, stop=False)
                    nc.tensor.matmul(vps[:], st[:, 1, t * 128:(t + 1) * 128],
                                     wvalS[:, 1, :], start=False, stop=False)
                    nc.tensor.matmul(vps[:], ones1S[:1, :], bvalS[:1, :],
                                     start=False, stop=True)
                    nc.scalar.activation(vb[:, t, :], vps[:], AF.Copy)
                nc.sync.dma_start(
                    _dap(vtab[:, :], [[D, 128], [128 * D, 2], [1, D]],
                         extra_off=prow * D),
                    vb[:])

            vwin = dataclasses.replace(
                vtab[:, :], ap=[[D, VROWS - W], [1, WIN]])

            # ================= BACK phase (needs vtab) ==================
            for m in range(NM):
                q0 = m * MACRO
                sTq, c3d = sTqs[m], c3ds[m]
                wxS = io.tile([128, NIDX // 16], i16, tag='wxS')
                nc.gpsimd.dma_start(
                    wxS[:], widx[:, m * (NIDX // 16):(m + 1) * (NIDX // 16)])
                tebS = io.tile([128, JT, D], fp16, tag='tebS')
                nc.gpsimd.dma_start(
                    tebS[:],
                    _dap(teb[:, :], [[D, 128], [128 * D, JT], [1, D]],
                         extra_off=q0 * D))

                gw = gat.tile([128, L * JT, WIN], fp16, tag='gw')
                half = NIDX // 2
                for g2 in range(2):
                    nc.gpsimd.dma_gather(
                        out_ap=gw[:, g2 * (L * JT // 2):(g2 + 1) * (L * JT // 2), :],
                        in_ap=vwin,
                        idxs_ap=wxS[:, g2 * (half // 16):(g2 + 1) * (half // 16)],
                        num_idxs=half, num_idxs_reg=half,
                        elem_size=WIN, elem_step=D)

                # sampling reduce per subtile
                attjs = []
                for j in range(JT):
                    tt = gat.tile([128, L * W * D], fp16, tag='tt', bufs=1)
                    nc.vector.tensor_tensor(
                        tt[:],
                        _bc(gw[:, :, :], [[JT * WIN, L], [D, W], [1, D]],
                            extra_off=j * WIN),
                        _bc(c3d[:, :], [[16, L * W], [0, 16], [1, 16]],
                            extra_off=j * 384), OP.mult)
                    Y = wk.tile([128, L * 3 * D], fp16, tag='Yf')
                    nc.vector.tensor_tensor(
                        Y[:],
                        _bc(tt[:, :], [[6 * D, L], [D, 3], [1, D]]),
                        _bc(tt[:, :], [[6 * D, L], [D, 3], [1, D]],
                            extra_off=3 * D), OP.add)
                    Zt = wk.tile([128, L * D], fp16, tag='Zf')
                    nc.vector.tensor_tensor(
                        Zt[:],
                        _bc(Y[:, :], [[3 * D, L], [1, D]]),
                        _bc(Y[:, :], [[3 * D, L], [1, D]], extra_off=D),
                        OP.add)
                    at0 = wk.tile([128, L * D], fp16, tag='at0')
                    nc.vector.tensor_tensor(
                        at0[:], Zt[:],
                        _bc(Y[:, :], [[3 * D, L], [1, D]], extra_off=2 * D),
                        OP.add)
                    s01 = wk.tile([128, 2 * D], fp16, tag='s01')
                    nc.vector.tensor_tensor(s01[:], at0[:, 0:2 * D],
                                            at0[:, 2 * D:4 * D], OP.add)
                    attj = wk.tile([128, D], fp16, tag='attj', bufs=3)
                    nc.vector.tensor_tensor(attj[:], s01[:, 0:D],
                                            s01[:, D:2 * D], OP.add)
                    attjs.append(attj)

                # out-projection + src residual (PSUM) + LN1
                xx = wk.tile([128, JT * D], fp16, tag='xx')
                xT = wk.tile([128, 2, MACRO], fp16, tag='xT')
                s2f = wk.tile([128, JT * D], fp16, tag='s2f')
                msJ = wk.tile([128, JT], f32, tag='msJ')
                ssqJ = wk.tile([128, JT], f32, tag='ssqJ')
                for j in range(JT):
                    attT = wk.tile([128, 2, 128], fp16, tag='attT', bufs=2)
                    tp2 = ptr.tile([128, 256], fp16, tag='tr')
                    for c in range(2):
                        nc.tensor.transpose(
                            tp2[:, c * 128:(c + 1) * 128],
                            attjs[j][:, c * 128:(c + 1) * 128],
                            identS[:])
                    nc.scalar.activation(
                        attT[:].rearrange("p a b -> p (a b)"), tp2[:], AF.Copy)
                    s2ps = p256.tile([128, D], f32, tag='p256')
                    nc.tensor.matmul(s2ps[:], attT[:, 0, :], woutS[:, 0, :],
                                     start=True, stop=False)
                    nc.tensor.matmul(s2ps[:], attT[:, 1, :], woutS[:, 1, :],
                                     start=False, stop=False)
                    for c in range(2):
                        nc.tensor.matmul(
                            s2ps[:],
                            sTq[:, c, j * 128:(j + 1) * 128],
                            iresS[:, c, :], start=False, stop=False)
                    nc.tensor.matmul(s2ps[:], ones1S[:1, :], boutS[:1, :],
                                     start=False, stop=True)
                    nc.scalar.activation(s2f[:, j * D:(j + 1) * D], s2ps[:],
                                         AF.Copy, accum_out=msJ[:, j:j + 1])
                    sq = wk.tile([128, D], fp16, tag='sq')
                    nc.scalar.activation(sq[:], s2ps[:], AF.Square,
                                         accum_out=ssqJ[:, j:j + 1])
                # batched LN1 stats (FD=JT)
                muJ = wk.tile([128, JT], f32, tag='muJ')
                nc.vector.tensor_scalar(muJ[:], msJ[:], 1.0 / D, None, OP.mult)
                mu2J = wk.tile([128, JT], f32, tag='mu2J')
                nc.vector.tensor_tensor(mu2J[:], muJ[:], muJ[:], OP.mult)
                varJ = wk.tile([128, JT], f32, tag='varJ')
                nc.vector.scalar_tensor_tensor(
                    varJ[:], ssqJ[:], 1.0 / D, mu2J[:], OP.mult, OP.subtract)
                lnvJ = wk.tile([128, JT], f32, tag='lnvJ')
                nc.scalar.activation(lnvJ[:], varJ[:], AF.Ln,
                                     bias=cbiasS[:, 6:7], scale=1.0)
                rsdJ = wk.tile([128, JT], f32, tag='rsdJ')
                nc.scalar.activation(rsdJ[:], lnvJ[:], AF.Exp, bias=0.0,
                                     scale=-0.5)
                nmrJ = wk.tile([128, JT], f32, tag='nmrJ')
                nc.vector.scalar_tensor_tensor(
                    nmrJ[:], muJ[:], -1.0, rsdJ[:], OP.mult, OP.mult)
                for j in range(JT):
                    xn = wk.tile([128, D], fp16, tag='xn')
                    nc.scalar.activation(xn[:], s2f[:, j * D:(j + 1) * D],
                                         AF.Identity, bias=nmrJ[:, j:j + 1],
                                         scale=rsdJ[:, j:j + 1])
                    if G1_ONE:
                        nc.vector.tensor_tensor(xx[:, j * D:(j + 1) * D],
                                                xn[:], tebS[:, j, :], OP.add)
                    else:
                        t1 = wk.tile([128, D], fp16, tag='t1')
                        nc.vector.tensor_tensor(t1[:], xn[:], ln1gS[:, :],
                                                OP.mult)
                        nc.vector.tensor_tensor(xx[:, j * D:(j + 1) * D],
                                                t1[:], tebS[:, j, :], OP.add)
                    # transpose xx_j for FFN + residual
                    tp3 = ptr.tile([128, 256], fp16, tag='tr')
                    for c in range(2):
                        nc.tensor.transpose(
                            tp3[:, c * 128:(c + 1) * 128],
                            xx[:, j * D + c * 128:j * D + (c + 1) * 128],
                            identS[:])
                    nc.scalar.activation(
                        _bc(xT[:, :, :], [[MACRO, 2], [1, 128]],
                            extra_off=j * 128), tp3[:], AF.Copy)

                # FFN w1 (all subtiles at once)
                hbT = wk.tile([128, 8, MACRO], fp16, tag='hbT')
                for k in range(8):
                    hps = php.tile([128, MACRO], f32, tag='hps')
                    nc.tensor.matmul(hps[:], w1S[:, 0, k * 128:(k + 1) * 128],
                                     xT[:, 0, :], start=True, stop=False)
                    nc.tensor.matmul(hps[:], w1S[:, 1, k * 128:(k + 1) * 128],
                                     xT[:, 1, :], start=False, stop=True)
                    nc.scalar.activation(hbT[:, k, :], hps[:], AF.Relu,
                                         bias=b1tS[:, k:k + 1])

                # FFN w2 + xx residual + LN2
                of = wk.tile([128, JT * D], fp16, tag='of', bufs=2)
                o2f = wk.tile([128, JT * D], fp16, tag='o2f')
                ms2J = wk.tile([128, JT], f32, tag='ms2J')
                ssq2J = wk.tile([128, JT], f32, tag='ssq2J')
                for j in range(JT):
                    o2ps = p256.tile([128, D], f32, tag='p256')
                    for k in range(8):
                        nc.tensor.matmul(
                            o2ps[:], hbT[:, k, j * 128:(j + 1) * 128],
                            w2S[:, k, :], start=(k == 0), stop=False)
                    for c in range(2):
                        nc.tensor.matmul(
                            o2ps[:], xT[:, c, j * 128:(j + 1) * 128],
                            iresS[:, c, :], start=False, stop=False)
                    nc.tensor.matmul(o2ps[:], ones1S[:1, :], b2S[:1, :],
                                     start=False, stop=True)
                    nc.scalar.activation(o2f[:, j * D:(j + 1) * D], o2ps[:],
                                         AF.Copy, accum_out=ms2J[:, j:j + 1])
                    sq2 = wk.tile([128, D], fp16, tag='sq2')
                    nc.scalar.activation(sq2[:], o2ps[:], AF.Square,
                                         accum_out=ssq2J[:, j:j + 1])
                mu_2J = wk.tile([128, JT], f32, tag='mu_2J')
                nc.vector.tensor_scalar(mu_2J[:], ms2J[:], 1.0 / D, None,
                                        OP.mult)
                mu22J = wk.tile([128, JT], f32, tag='mu22J')
                nc.vector.tensor_tensor(mu22J[:], mu_2J[:], mu_2J[:], OP.mult)
                var2J = wk.tile([128, JT], f32, tag='var2J')
                nc.vector.scalar_tensor_tensor(
                    var2J[:], ssq2J[:], 1.0 / D, mu22J[:], OP.mult,
                    OP.subtract)
                lnv2J = wk.tile([128, JT], f32, tag='lnv2J')
                nc.scalar.activation(lnv2J[:], var2J[:], AF.Ln,
                                     bias=cbiasS[:, 6:7], scale=1.0)
                rsd2J = wk.tile([128, JT], f32, tag='rsd2J')
                nc.scalar.activation(rsd2J[:], lnv2J[:], AF.Exp, bias=0.0,
                                     scale=-0.5)
                nmr2J = wk.tile([128, JT], f32, tag='nmr2J')
                nc.vector.scalar_tensor_tensor(
                    nmr2J[:], mu_2J[:], -1.0, rsd2J[:], OP.mult, OP.mult)
                for j in range(JT):
                    if G2_ONE and B2_ZERO:
                        nc.scalar.activation(
                            of[:, j * D:(j + 1) * D],
                            o2f[:, j * D:(j + 1) * D],
                            AF.Identity, bias=nmr2J[:, j:j + 1],
                            scale=rsd2J[:, j:j + 1])
                    else:
                        xn2 = wk.tile([128, D], fp16, tag='xn2')
                        nc.scalar.activation(
                            xn2[:], o2f[:, j * D:(j + 1) * D],
                            AF.Identity, bias=nmr2J[:, j:j + 1],
                            scale=rsd2J[:, j:j + 1])
                        t2 = wk.tile([128, D], fp16, tag='t2')
                        nc.vector.tensor_tensor(t2[:], xn2[:], ln2gS[:, :],
                                                OP.mult)
                        nc.vector.tensor_tensor(of[:, j * D:(j + 1) * D],
                                                t2[:], ln2bS[:, :], OP.add)

                nc.gpsimd.dma_start(
                    _dap(out_q[:, :], [[D, 128], [128 * D, JT], [1, D]],
                         extra_off=q0 * D),
                    of[:])

    if os.environ.get('K_NOSPLIT', '0') != '1':
        _split_excess_waits(nc)
    lower_extended_insts(nc)
    return nc


_PROG = None
LAST_RESULTS = None
_FLAGS = {'g1_one': False, 'g2_one': False, 'b2_zero': False}


def _get_program():
    global _PROG
    if _PROG is None:
        _PROG = _build_program()
    return _PROG


# column permutation: new col (k2, h, u) <- old col h*32 + k2*2 + u
_PERM = np.zeros(D, np.int64)
for _k2 in range(16):
    for _h in range(H):
        for _u in range(2):
            _PERM[_k2 * 16 + _h * 2 + _u] = _h * 32 + _k2 * 2 + _u

# off/attn projection column permutation: new col (l, p, h) <- old (h, l, p)
_OAPERM = np.zeros(128, np.int64)
for _l in range(L):
    for _p in range(NP):
        for _h in range(H):
            _OAPERM[_l * 32 + _p * 8 + _h] = _h * 16 + _l * 4 + _p


def _host_consts():
    c = {}
    c['ident'] = np.eye(128, dtype=np.float16)
    c['ones1'] = np.ones((1, 128), np.float16)
    ir = np.zeros((128, 2, D), np.float16)
    for p in range(128):
        ir[p, 0, p] = 1.0
        ir[p, 1, 128 + p] = 1.0
    c['ires'] = ir
    return c


def _kchunk(w):
    """[256, X] f32 -> [128, 2, X] fp16"""
    w = np.asarray(w, np.float32)
    return np.stack([w[0:128], w[128:256]], axis=1).astype(np.float16)


def kernel(src, pos, time_embed, reference_points, w_off, b_off, w_attn,
           b_attn, w_val, b_val, w_out, b_out, ln1_g, ln1_b, w1, b1, w2, b2,
           ln2_g, ln2_b, spatial_shapes, level_start_index):
    src = np.asarray(src, np.float32)
    pos = np.asarray(pos, np.float32)
    te = np.asarray(time_embed, np.float32)
    ref = np.asarray(reference_points, np.float32).reshape(2, LEN_IN, L)

    consts = _host_consts()
    _FLAGS['g1_one'] = bool(np.allclose(np.asarray(ln1_g, np.float32), 1.0))
    _FLAGS['g2_one'] = bool(np.allclose(np.asarray(ln2_g, np.float32), 1.0))
    _FLAGS['b2_zero'] = bool(np.allclose(np.asarray(ln2_b, np.float32), 0.0))

    woa_full = np.concatenate([np.asarray(w_off, np.float32)[:, _OAPERM],
                               np.asarray(w_attn, np.float32)[:, _OAPERM]],
                              axis=1)
    boa_full = np.concatenate([np.asarray(b_off, np.float32)[_OAPERM],
                               np.asarray(b_attn, np.float32)[_OAPERM]])[None, :]

    wval_p = np.asarray(w_val, np.float32)[:, _PERM]
    wout_r = np.asarray(w_out, np.float32)[_PERM, :]

    w2f = np.asarray(w2, np.float32)
    w2c = np.stack([w2f[k * 128:(k + 1) * 128] for k in range(8)],
                   axis=1).astype(np.float16)
    b1f = np.asarray(b1, np.float32)
    b1t = np.stack([b1f[k * 128:(k + 1) * 128] for k in range(8)], axis=1)
    rep16 = lambda v: np.repeat(
        np.asarray(v, np.float32)[None, :], 128, axis=0).astype(np.float16)

    shared = {
        'wvalP': _kchunk(wval_p),
        'bvalP': np.asarray(b_val, np.float32)[_PERM][None, :].astype(np.float16),
        'woa': _kchunk(woa_full),
        'boa': boa_full.astype(np.float16),
        'woutR': _kchunk(wout_r),
        'bout': np.asarray(b_out, np.float32)[None, :].astype(np.float16),
        'w1': _kchunk(np.asarray(w1, np.float32)),
        'b1t': b1t.astype(np.float32),
        'w2': w2c,
        'b2': np.asarray(b2, np.float32)[None, :].astype(np.float16),
        'ln1g': rep16(ln1_g),
        'ln2g': rep16(ln2_g),
        'ln2b': rep16(ln2_b),
        'cbias': np.repeat(np.array([[0., -1., -2., -3., -4., -5., EPS, 0.]],
                                    np.float32), 128, axis=0),
        **consts,
    }

    ln1b_f = np.asarray(ln1_b, np.float32)
    tscale = np.array(LEVEL_LENS, np.float32)

    in_maps = []
    for core in range(N_CORES):
        n, qr = core // 4, core % 4
        qs = slice(qr * QC, (qr + 1) * QC)
        m = dict(shared)
        # src^T k-chunked, full batch
        sT = src[n].T  # [256, 15360]
        m['srcTk'] = np.stack([sT[0:128], sT[128:256]], axis=1).astype(np.float16)
        m['srcTq'] = np.ascontiguousarray(m['srcTk'][:, :, qs])
        pT = pos[n, qs].T
        m['posTk'] = np.stack([pT[0:128], pT[128:256]], axis=1).astype(np.float16)
        m['teb'] = (te[n, qs] + ln1b_f[None, :]).astype(np.float16)

        ar = ref[n, qs] * tscale[None, :] - np.float32(0.5)  # [3840, 4] f32
        flr = np.floor(ar)
        m['arfq'] = (ar - flr + np.float32(2.0)).astype(np.float32)
        rows = (flr.astype(np.int64)
                + np.array(PSTARTS, np.int64)[None, :] - 2)  # [3840, 4]
        # gather idx order per macro: i = l*384 + j*128 + p
        wx = np.zeros((16, NMAC * (NIDX // 16)), np.int16)
        ridx = rows.reshape(NMAC, JT, 128, L)  # [m, j, p, l]
        for mm_ in range(NMAC):
            blk = np.transpose(ridx[mm_], (2, 0, 1)).reshape(NIDX)  # (l, j, p)
            wx[:, mm_ * (NIDX // 16):(mm_ + 1) * (NIDX // 16)] = (
                blk.reshape(NIDX // 16, 16).T.astype(np.int16))
        m['widx'] = np.tile(wx, (8, 1))
        in_maps.append(m)

    nc = _get_program()
    from concourse.bass_utils import run_bass_kernel_spmd
    res = run_bass_kernel_spmd(nc, in_maps, core_ids=list(range(N_CORES)))
    global LAST_RESULTS
    LAST_RESULTS = res
    if getattr(res, 'exec_time_ns', None):
        print('HW exec time:', res.exec_time_ns, 'ns')

    out = np.zeros((2, LEN_IN, D), np.float32)
    for core in range(N_CORES):
        n, qr = core // 4, core % 4
        out[n, qr * QC:(qr + 1) * QC] = res.results[core]['out_q'].astype(np.float32)
    return out
